# revision 21
# baseline (speedup 1.0000x reference)
"""Causal self-attention (QK-RMSNorm + RoPE) on 8 Trainium2 NeuronCores.

Problem: x[2,2048,2048], Wq/Wk/Wv/Wo [2048,2048], 16 heads, head_dim 128.

Sharding: core c handles batch b=c//4 and head group g=c%4 (4 heads,
model cols [512g:512g+512)).

Single fused pipeline, one pass over x per core:
- Q/K are projected directly into transposed [head_dim, tokens] layout
  by making the weight tile the stationary matmul operand (no PE
  transposes, no DRAM roundtrip).  V is projected in [tokens, cols]
  layout for the AV matmul.
- RMS-norm uses a ones[128,128] matmul to produce the per-token sum of
  squares broadcast across all partitions in one shot; normalization is
  a single DVE divide.  RoPE runs on 64-partition halves against a
  transposed cos/sin table.
- Attention per 512-token chunk uses transposed scores
  (eT = exp(scale*kT.T@qT - 1)); the softmax denominator is accumulated
  on the vector engine (csum += eT) and turned into a broadcast
  denominator with one ones-matmul per (head, chunk); yt = yt_acc / den.
- Per-chunk AllGather over the 4 cores of each batch (not all 8), then
  each core computes a 512-row slice of yT = Wo @ yt_full, interleaved
  at two chunks of lag so collectives hide under compute.
"""

import math
from contextlib import ExitStack

import numpy as np

import bass_rust as _bass_rust

import concourse.bass as bass
import concourse.bacc as bacc
import concourse.tile as tile
from concourse import mybir
from concourse.bass_utils import run_bass_kernel_spmd
from concourse.hw_specs import get_activation_tables

P = 128
D = 2048
S = 2048
HD = 128              # head dim
NHL = 4               # heads per core
GW = NHL * HD         # 512, per-core width of head group
CT = D // P           # 16 contraction tiles
NTCH = 4              # token chunks of 512
NCORES = 8
F32 = mybir.dt.float32
F16 = mybir.dt.float16
F32R = mybir.dt.float32r
SCALE = 1.0 / math.sqrt(HD)
EPS = 1.1920928955078125e-07

_program_cache = {}

# All scalar-engine transcendentals here are exp/ln/copy; route every one of
# them to the single ACT table set that contains them all so the table is
# loaded exactly once (the default chooser picks the first covering set per
# function, which alternates sets and costs ~2.7us per switch).
_SET_WITH_ALL = "natural_log_exp_and_others"
_SHARED_FNS = {
    mybir.ActivationFunctionType.Exp,
    mybir.ActivationFunctionType.Ln,
    mybir.ActivationFunctionType.Copy,
}


class _Bacc(bacc.Bacc):
    def insert_act_table_loads(self):
        has_activation = any(
            isinstance(i, mybir.InstActivation)
            for b in self.main_func.blocks
            for i in b.instructions
        )
        if not has_activation:
            return
        tables = []
        for name, fns in get_activation_tables(self.m.arch).items():
            if name != _SET_WITH_ALL:
                fns = fns - _SHARED_FNS
            tables.append((name, fns))
        _bass_rust.insert_act_table_loads(self, tables)


def build_program():
    if "nc" in _program_cache:
        return _program_cache["nc"]

    nc = _Bacc("TRN2", target_bir_lowering=False, debug=False, num_devices=NCORES)

    xt_in = nc.dram_tensor("xt", [D, S], F16, kind="ExternalInput")
    wq_in = nc.dram_tensor("wq", [D, GW], F16, kind="ExternalInput")
    wk_in = nc.dram_tensor("wk", [D, GW], F16, kind="ExternalInput")
    wv_in = nc.dram_tensor("wv", [D, GW], F16, kind="ExternalInput")
    wo_in = nc.dram_tensor("wo", [D, GW], F16, kind="ExternalInput")
    cs_in = nc.dram_tensor("cs", [P, 2, S], F16, kind="ExternalInput")
    mask_in = nc.dram_tensor("maskt", [4, P, 512], F16, kind="ExternalInput")
    yt_out = nc.dram_tensor("yt_out", [GW, S], F32, kind="ExternalOutput")

    with tile.TileContext(nc) as tc:
        with ExitStack() as ctx:
            const = ctx.enter_context(tc.tile_pool(name="const", bufs=1))
            dram = ctx.enter_context(tc.tile_pool(name="dram", bufs=1, space="DRAM"))

            eps_t = const.tile([P, 1], F32, name="eps_t")
            nc.vector.memset(eps_t[:], EPS)
            neg1_t = const.tile([P, 1], F32, name="neg1_t")
            nc.vector.memset(neg1_t[:], -1.0)
            ones_h = const.tile([P, P], F16, name="ones_h")
            nc.vector.memset(ones_h[:], 1.0)

            # plane 0: cos duplicated on both partition halves; plane 1:
            # +sin on rows 0..63, -sin on rows 64..127 (rope sign folded)
            cs_sb = const.tile([P, 2, S], F16, name="cs_sb")
            nc.sync.dma_start(out=cs_sb[:], in_=cs_in[:, :, :])
            mask_sb = const.tile([P, 4, 512], F16, name="mask_sb")
            nc.sync.dma_start(out=mask_sb[:], in_=mask_in.ap().rearrange("t p f -> p t f"))

            # attention sub-chunks (q0, width); the last 512 tokens are
            # split so the final AllGather is small and hides under o_proj
            chunks = [(0, 512), (512, 512), (1024, 512), (1536, 256), (1792, 256)]
            yt_ics = [
                dram.tile([GW, w], F16, name=f"yt_ic{i}")
                for i, (q0, w) in enumerate(chunks)
            ]
            ag_ics = [
                dram.tile([4 * GW, w], F16, name=f"ag_ic{i}")
                for i, (q0, w) in enumerate(chunks)
            ]

            # persistent SBUF
            wpool = ctx.enter_context(tc.tile_pool(name="wpool", bufs=1))
            wq_sb = wpool.tile([P, CT, GW], F16, name="wq_sb")
            wk_sb = wpool.tile([P, CT, GW], F16, name="wk_sb")
            wv_sb = wpool.tile([P, CT, GW], F16, name="wv_sb")
            wo_sb = wpool.tile([P, CT, GW], F16, name="wo_sb")
            qkv = ctx.enter_context(tc.tile_pool(name="qkv", bufs=1))
            qt_sb = qkv.tile([P, NHL, S], F16, name="qt_sb")
            kt_sb = qkv.tile([P, NHL, S], F16, name="kt_sb")
            v_sb = qkv.tile([P, CT, GW], F16, name="v_sb")

            # streaming pools (x in 256-token half-chunks)
            xtp = ctx.enter_context(tc.tile_pool(name="xtp", bufs=3))
            rawp = ctx.enter_context(tc.tile_pool(name="rawp", bufs=2))
            sqp = ctx.enter_context(tc.tile_pool(name="sqp", bufs=3))
            nrmp = ctx.enter_context(tc.tile_pool(name="nrmp", bufs=2))
            qsp = ctx.enter_context(tc.tile_pool(name="qsp", bufs=2))
            mp = ctx.enter_context(tc.tile_pool(name="mp", bufs=2))
            etp = ctx.enter_context(tc.tile_pool(name="etp", bufs=4))
            denp = ctx.enter_context(tc.tile_pool(name="denp", bufs=2))
            ytsp = ctx.enter_context(tc.tile_pool(name="ytsp", bufs=2))
            agp = ctx.enter_context(tc.tile_pool(name="agp", bufs=2))
            ysp = ctx.enter_context(tc.tile_pool(name="ysp", bufs=2))

            # PSUM: 2+2+2+2 = 8 banks
            proj_ps = ctx.enter_context(tc.tile_pool(name="proj_ps", bufs=2, space="PSUM"))
            s_ps = ctx.enter_context(tc.tile_pool(name="s_ps", bufs=2, space="PSUM"))
            yt_ps = ctx.enter_context(tc.tile_pool(name="yt_ps", bufs=2, space="PSUM"))
            bc_ps = ctx.enter_context(tc.tile_pool(name="bc_ps", bufs=2, space="PSUM"))

            # weight loads: wq per-ct on sync (interleaved with x chunk 0
            # below); wk/wv/wo as single rearranged DMAs on scalar
            nc.scalar.dma_start(
                out=wk_sb[:], in_=wk_in.ap().rearrange("(a p) f -> p a f", p=P))
            nc.scalar.dma_start(
                out=wv_sb[:], in_=wv_in.ap().rearrange("(a p) f -> p a f", p=P))
            nc.scalar.dma_start(
                out=wo_sb[:], in_=wo_in.ap().rearrange("(a p) f -> p a f", p=P))

            def emit_oproj(icc):
                q0, w = chunks[icc]
                ag_a = agp.tile([P, 8, 512], F16, name=f"ag_a{icc}", tag="ag")
                ag_b = agp.tile([P, 8, 512], F16, name=f"ag_b{icc}", tag="ag")
                for half, agt in ((0, ag_a), (1, ag_b)):
                    for m4 in range(2):
                        mt = half * 8 + m4 * 4
                        nc.sync.dma_start(
                            out=agt[:, m4 * 4:m4 * 4 + 4, 0:w],
                            in_=ag_ics[icc][mt * P:(mt + 4) * P, :]
                                .rearrange("(a p) f -> p a f", p=P),
                        )
                for oc in range(4):
                    yp = proj_ps.tile([P, w], F32, name=f"yp{icc}_{oc}", tag="proj")
                    for mt in range(CT):
                        agt = ag_a if mt < 8 else ag_b
                        nc.tensor.matmul(
                            yp[:],
                            wo_sb[:, mt, oc * P:(oc + 1) * P],
                            agt[:, mt % 8, 0:w],
                            start=(mt == 0), stop=(mt == CT - 1),
                        )
                    y_sb = ysp.tile([P, w], F32, name=f"ysb{icc}_{oc}", tag="ysb")
                    nc.scalar.copy(y_sb[:], yp[:])
                    nc.scalar.dma_start(
                        out=yt_out[oc * P:(oc + 1) * P, q0:q0 + w],
                        in_=y_sb[:],
                    )

            def emit_att(ci):
                q0, w = chunks[ci]
                njb = (q0 + w) // P
                jb0 = q0 // P
                for h in range(NHL):
                    ytp = yt_ps.tile([P, w], F32, name=f"yt{ci}_{h}", tag="yt")
                    den = bc_ps.tile([P, w], F32, name=f"den{ci}_{h}", tag="bc")
                    ets = [None] * njb
                    for jb in range(njb):
                        sp = s_ps.tile([P, w], F32, name=f"s{ci}_{h}_{jb}", tag="s")
                        nc.tensor.matmul(
                            sp[:],
                            kt_sb[:, h, jb * P:(jb + 1) * P],
                            qt_sb[:, h, q0:q0 + w],
                            start=True, stop=True,
                        )
                        et = etp.tile([P, w], F16, name=f"et{ci}_{h}_{jb}", tag="et")
                        nc.scalar.activation(
                            et[:], sp[:],
                            mybir.ActivationFunctionType.Exp,
                            bias=neg1_t[:], scale=SCALE,
                        )
                        t = jb - jb0
                        if t >= 0:
                            nc.vector.tensor_mul(et[:], et[:], mask_sb[:, t, 0:w])
                        ets[jb] = et
                        # AV and the ones-matmul denominator accumulation lag
                        # the score by one tile so PE never waits on exp
                        if jb >= 1:
                            nc.tensor.matmul(
                                ytp[:],
                                v_sb[:, jb - 1, h * HD:(h + 1) * HD],
                                ets[jb - 1][:],
                                start=(jb - 1 == 0), stop=False,
                            )
                            nc.tensor.matmul(
                                den[:], ones_h[:], ets[jb - 1][:],
                                start=(jb - 1 == 0), stop=False,
                            )
                    nc.tensor.matmul(
                        ytp[:],
                        v_sb[:, njb - 1, h * HD:(h + 1) * HD],
                        ets[njb - 1][:],
                        start=(njb == 1), stop=True,
                    )
                    nc.tensor.matmul(
                        den[:], ones_h[:], ets[njb - 1][:],
                        start=(njb == 1), stop=True,
                    )
                    # rden = exp(-ln(den)) on the scalar engine (same ACT
                    # table set as the softmax exp)
                    lnd = denp.tile([P, w], F32, name=f"lnd{ci}_{h}", tag="lnd", bufs=1)
                    nc.scalar.activation(
                        lnd[:], den[:], mybir.ActivationFunctionType.Ln)
                    rden = denp.tile([P, w], F32, name=f"rdn{ci}_{h}", tag="rden")
                    nc.scalar.activation(
                        rden[:], lnd[:], mybir.ActivationFunctionType.Exp,
                        scale=-1.0)
                    yt_sb = ytsp.tile([P, w], F16, name=f"yts{ci}_{h}", tag="yts")
                    nc.vector.tensor_mul(yt_sb[:], ytp[:], rden[:])
                    nc.scalar.dma_start(
                        out=yt_ics[ci][h * P:(h + 1) * P, :], in_=yt_sb[:])

                nc.gpsimd.collective_compute(
                    "AllGather",
                    mybir.AluOpType.bypass,
                    replica_groups=[[0, 1, 2, 3], [4, 5, 6, 7]],
                    ins=[yt_ics[ci][:].opt()],
                    outs=[ag_ics[ci][:].opt()],
                )

            for tch in range(NTCH):
                tc0 = tch * 512
                for half in range(2):
                    hc0 = tc0 + half * 256
                    xt_ch = xtp.tile(
                        [P, CT, 256], F16, name=f"xt{tch}_{half}", tag="xt")
                    for c4 in range(4):
                        ct = c4 * 4
                        if tch == 0 and half == 0:
                            nc.sync.dma_start(
                                out=wq_sb[:, ct:ct + 4, :],
                                in_=wq_in[ct * P:(ct + 4) * P, :]
                                    .rearrange("(a p) f -> p a f", p=P),
                            )
                        nc.sync.dma_start(
                            out=xt_ch[:, ct:ct + 4, :],
                            in_=xt_in[ct * P:(ct + 4) * P, hc0:hc0 + 256]
                                .rearrange("(a p) f -> p a f", p=P),
                        )

                    # ---- Q then K: transposed projection + rms-norm + rope ----
                    for wsb, dst, tag in ((wq_sb, qt_sb, "q"), (wk_sb, kt_sb, "k")):
                        raw4 = rawp.tile(
                            [P, NHL, 256], F16, name=f"{tag}raw{tch}_{half}", tag="raw")
                        nrm4 = nrmp.tile(
                            [P, NHL, 256], F16, name=f"{tag}nrm{tch}_{half}", tag="nrm")
                        sqs = []
                        for h in range(NHL):
                            ps = proj_ps.tile(
                                [P, 256], F32, name=f"{tag}ps{tch}_{half}_{h}",
                                tag="proj")
                            for ct in range(CT):
                                nc.tensor.matmul(
                                    ps[:],
                                    wsb[:, ct, h * P:(h + 1) * P],
                                    xt_ch[:, ct, :],
                                    start=(ct == 0), stop=(ct == CT - 1),
                                )
                            nc.vector.tensor_copy(raw4[:, h, :], ps[:])
                            sq = sqp.tile(
                                [P, 256], F16, name=f"{tag}sq{tch}_{half}_{h}",
                                tag="sq")
                            nc.vector.tensor_mul(sq[:], raw4[:, h, :], raw4[:, h, :])
                            sqs.append(sq)
                        # partition-swapped copy of raw4 (q2 on rows 0..63)
                        qs4 = qsp.tile(
                            [P, NHL, 256], F16, name=f"{tag}qs{tch}_{half}", tag="qs")
                        nc.sync.dma_start(out=qs4[0:64, :, :], in_=raw4[64:128, :, :])
                        nc.sync.dma_start(out=qs4[64:128, :, :], in_=raw4[0:64, :, :])
                        for h in range(NHL):
                            ssum = bc_ps.tile(
                                [P, 256], F32, name=f"{tag}ss{tch}_{half}_{h}",
                                tag="bc")
                            nc.tensor.matmul(
                                ssum[:], ones_h[:], sqs[h][:], start=True, stop=True)
                            # rstd = exp(-0.5*ln(ms+eps)) — Ln and Exp share
                            # one ACT table set, so no table switches
                            lnt = sqp.tile(
                                [P, 256], F16, name=f"{tag}ln{tch}_{half}_{h}",
                                tag="lnt")
                            nc.scalar.activation(
                                lnt[:], ssum[:],
                                mybir.ActivationFunctionType.Ln,
                                bias=eps_t[:], scale=1.0 / HD,
                            )
                            nc.scalar.activation(
                                nrm4[:, h, :], lnt[:],
                                mybir.ActivationFunctionType.Exp,
                                scale=-0.5,
                            )
                        # rope: m1 = raw*cos_dup; m2 = swapped*sin_signed;
                        # dst = (m1 + m2) * rstd
                        cosB = cs_sb[:, 0:1, hc0:hc0 + 256].broadcast_to((P, NHL, 256))
                        sinB = cs_sb[:, 1:2, hc0:hc0 + 256].broadcast_to((P, NHL, 256))
                        m1 = mp.tile(
                            [P, NHL, 256], F16, name=f"{tag}m1{tch}_{half}", tag="m1")
                        m2 = mp.tile(
                            [P, NHL, 256], F16, name=f"{tag}m2{tch}_{half}", tag="m2")
                        nc.vector.tensor_mul(m1[:], raw4[:], cosB)
                        nc.vector.tensor_mul(m2[:], qs4[:], sinB)
                        nc.vector.tensor_add(m1[:], m1[:], m2[:])
                        nc.vector.tensor_mul(
                            dst[:, :, hc0:hc0 + 256], m1[:], nrm4[:])

                    # ---- V: row-layout projection ----
                    for ib in range(2):
                        jb = tch * 4 + half * 2 + ib
                        ps = proj_ps.tile([P, GW], F32, name=f"vps{jb}", tag="proj")
                        for ct in range(CT):
                            nc.tensor.matmul(
                                ps[:],
                                xt_ch[:, ct, ib * P:(ib + 1) * P],
                                wv_sb[:, ct, :],
                                start=(ct == 0), stop=(ct == CT - 1),
                            )
                        nc.vector.tensor_copy(v_sb[:, jb, :], ps[:])

                    if tch == 3:
                        emit_att(3 if half == 0 else 4)

                if tch < 3:
                    emit_att(tch)
                if tch == 2:
                    emit_oproj(0)

            emit_oproj(1)
            emit_oproj(2)
            emit_oproj(3)
            emit_oproj(4)

    nc.compile()
    _program_cache["nc"] = nc
    return nc


def _rope_tables():
    inv_freq = 1.0 / (10000.0 ** (np.arange(0, HD, 2, dtype=np.float32) / HD))
    pos = np.arange(S, dtype=np.float32)
    freqs = np.outer(pos, inv_freq).astype(np.float32)  # [S, 64]
    cosT = np.cos(freqs).T  # [64, S]
    sinT = np.sin(freqs).T
    cs = np.empty((P, 2, S), dtype=np.float16)
    cs[0:64, 0] = cosT
    cs[64:128, 0] = cosT
    cs[0:64, 1] = sinT
    cs[64:128, 1] = -sinT
    return cs


def _mask_tiles():
    m = np.zeros((4, P, 512), dtype=np.float16)
    jj = np.arange(P)[:, None]
    ii = np.arange(512)[None, :]
    for t in range(4):
        m[t] = np.where(t * P + jj > ii, 0.0, 1.0)
    return m


def make_in_maps(x, Wq, Wk, Wv, Wo):
    x = np.asarray(x, dtype=np.float32)
    cs = _rope_tables()
    maskt = _mask_tiles()
    wqT = np.ascontiguousarray(np.asarray(Wq, dtype=np.float32).T.astype(np.float16))
    wkT = np.ascontiguousarray(np.asarray(Wk, dtype=np.float32).T.astype(np.float16))
    wvT = np.ascontiguousarray(np.asarray(Wv, dtype=np.float32).T.astype(np.float16))
    woT = np.ascontiguousarray(np.asarray(Wo, dtype=np.float32).T.astype(np.float16))
    xts = [np.ascontiguousarray(x[b].T.astype(np.float16)) for b in range(2)]
    in_maps = []
    for c in range(NCORES):
        b, g = c // 4, c % 4
        sl = slice(g * GW, (g + 1) * GW)
        in_maps.append({
            "xt": xts[b],
            "wq": np.ascontiguousarray(wqT[:, sl]),
            "wk": np.ascontiguousarray(wkT[:, sl]),
            "wv": np.ascontiguousarray(wvT[:, sl]),
            "wo": np.ascontiguousarray(woT[:, sl]),
            "cs": cs,
            "maskt": maskt,
        })
    return in_maps


def assemble_output(results):
    y = np.empty((2, S, D), dtype=np.float32)
    for c in range(NCORES):
        b, g = c // 4, c % 4
        y[b][:, g * GW:(g + 1) * GW] = results[c]["yt_out"].T
    return y


def kernel(x, Wq, Wk, Wv, Wo):
    nc = build_program()
    in_maps = make_in_maps(x, Wq, Wk, Wv, Wo)
    res = run_bass_kernel_spmd(nc, in_maps, core_ids=list(range(NCORES)))
    return assemble_output(res.results)


# revision 24
# speedup vs baseline: 1.0925x; 1.0925x over previous
"""Causal self-attention (QK-RMSNorm + RoPE) on 8 Trainium2 NeuronCores.

Problem: x[2,2048,2048], Wq/Wk/Wv/Wo [2048,2048], 16 heads, head_dim 128.

Sharding: core c handles batch b=c//4 and head group g=c%4 (4 heads,
model cols [512g:512g+512)).

Single fused pipeline, one pass over x per core:
- Q/K are projected directly into transposed [head_dim, tokens] layout
  by making the weight tile the stationary matmul operand (no PE
  transposes, no DRAM roundtrip).  V is projected in [tokens, cols]
  layout for the AV matmul.
- RMS-norm uses a ones[128,128] matmul to produce the per-token sum of
  squares broadcast across all partitions in one shot; normalization is
  a single DVE divide.  RoPE runs on 64-partition halves against a
  transposed cos/sin table.
- Attention per 512-token chunk uses transposed scores
  (eT = exp(scale*kT.T@qT - 1)); the softmax denominator is accumulated
  on the vector engine (csum += eT) and turned into a broadcast
  denominator with one ones-matmul per (head, chunk); yt = yt_acc / den.
- Per-chunk AllGather over the 4 cores of each batch (not all 8), then
  each core computes a 512-row slice of yT = Wo @ yt_full, interleaved
  at two chunks of lag so collectives hide under compute.
"""

import math
from contextlib import ExitStack

import numpy as np

import bass_rust as _bass_rust

import concourse.bass as bass
import concourse.bacc as bacc
import concourse.tile as tile
from concourse import mybir
from concourse.bass_utils import run_bass_kernel_spmd
from concourse.hw_specs import get_activation_tables

P = 128
D = 2048
S = 2048
HD = 128              # head dim
NHL = 4               # heads per core
GW = NHL * HD         # 512, per-core width of head group
CT = D // P           # 16 contraction tiles
NTCH = 4              # token chunks of 512
NCORES = 8
F32 = mybir.dt.float32
F16 = mybir.dt.float16
F32R = mybir.dt.float32r
SCALE = 1.0 / math.sqrt(HD)
EPS = 1.1920928955078125e-07

_program_cache = {}

# All scalar-engine transcendentals here are exp/ln/copy; route every one of
# them to the single ACT table set that contains them all so the table is
# loaded exactly once (the default chooser picks the first covering set per
# function, which alternates sets and costs ~2.7us per switch).
_SET_WITH_ALL = "natural_log_exp_and_others"
_SHARED_FNS = {
    mybir.ActivationFunctionType.Exp,
    mybir.ActivationFunctionType.Ln,
    mybir.ActivationFunctionType.Copy,
}


class _Bacc(bacc.Bacc):
    def insert_act_table_loads(self):
        has_activation = any(
            isinstance(i, mybir.InstActivation)
            for b in self.main_func.blocks
            for i in b.instructions
        )
        if not has_activation:
            return
        tables = []
        for name, fns in get_activation_tables(self.m.arch).items():
            if name != _SET_WITH_ALL:
                fns = fns - _SHARED_FNS
            tables.append((name, fns))
        _bass_rust.insert_act_table_loads(self, tables)


def build_program():
    if "nc" in _program_cache:
        return _program_cache["nc"]

    nc = _Bacc("TRN2", target_bir_lowering=False, debug=False, num_devices=NCORES)

    xt_in = nc.dram_tensor("xt", [D, S], F16, kind="ExternalInput")
    wq_in = nc.dram_tensor("wq", [D, GW], F16, kind="ExternalInput")
    wk_in = nc.dram_tensor("wk", [D, GW], F16, kind="ExternalInput")
    wv_in = nc.dram_tensor("wv", [D, GW], F16, kind="ExternalInput")
    wo_in = nc.dram_tensor("wo", [D, GW], F16, kind="ExternalInput")
    cs_in = nc.dram_tensor("cs", [P, 2, S], F16, kind="ExternalInput")
    mask_in = nc.dram_tensor("maskt", [4, P, 512], F16, kind="ExternalInput")
    psw_in = nc.dram_tensor("psw", [P, P], F16, kind="ExternalInput")
    yt_out = nc.dram_tensor("yt_out", [GW, S], F32, kind="ExternalOutput")

    with tile.TileContext(nc) as tc:
        with ExitStack() as ctx:
            const = ctx.enter_context(tc.tile_pool(name="const", bufs=1))
            dram = ctx.enter_context(tc.tile_pool(name="dram", bufs=1, space="DRAM"))

            eps_t = const.tile([P, 1], F32, name="eps_t")
            nc.vector.memset(eps_t[:], EPS)
            neg1_t = const.tile([P, 1], F32, name="neg1_t")
            nc.vector.memset(neg1_t[:], -1.0)
            ones_h = const.tile([P, P], F16, name="ones_h")
            nc.vector.memset(ones_h[:], 1.0)

            # plane 0: cos duplicated on both partition halves; plane 1:
            # +sin on rows 0..63, -sin on rows 64..127 (rope sign folded)
            cs_sb = const.tile([P, 2, S], F16, name="cs_sb")
            nc.sync.dma_start(out=cs_sb[:], in_=cs_in[:, :, :])
            mask_sb = const.tile([P, 4, 512], F16, name="mask_sb")
            nc.sync.dma_start(out=mask_sb[:], in_=mask_in.ap().rearrange("t p f -> p t f"))
            psw_sb = const.tile([P, P], F16, name="psw_sb")
            nc.scalar.dma_start(out=psw_sb[:], in_=psw_in[:, :])

            # attention sub-chunks (q0, width); the last 512 tokens are
            # split so the final AllGather is small and hides under o_proj
            chunks = [(0, 512), (512, 512), (1024, 512), (1536, 256), (1792, 256)]
            yt_ics = [
                dram.tile([GW, w], F16, name=f"yt_ic{i}")
                for i, (q0, w) in enumerate(chunks)
            ]
            ag_ics = [
                dram.tile([4 * GW, w], F16, name=f"ag_ic{i}")
                for i, (q0, w) in enumerate(chunks)
            ]

            # persistent SBUF
            wpool = ctx.enter_context(tc.tile_pool(name="wpool", bufs=1))
            wq_sb = wpool.tile([P, CT, GW], F16, name="wq_sb")
            wk_sb = wpool.tile([P, CT, GW], F16, name="wk_sb")
            wv_sb = wpool.tile([P, CT, GW], F16, name="wv_sb")
            wo_sb = wpool.tile([P, CT, GW], F16, name="wo_sb")
            qkv = ctx.enter_context(tc.tile_pool(name="qkv", bufs=1))
            qt_sb = qkv.tile([P, NHL, S], F16, name="qt_sb")
            kt_sb = qkv.tile([P, NHL, S], F16, name="kt_sb")
            v_sb = qkv.tile([P, CT, GW], F16, name="v_sb")

            # streaming pools (x in 256-token half-chunks)
            xtp = ctx.enter_context(tc.tile_pool(name="xtp", bufs=3))
            rawp = ctx.enter_context(tc.tile_pool(name="rawp", bufs=2))
            sqp = ctx.enter_context(tc.tile_pool(name="sqp", bufs=3))
            nrmp = ctx.enter_context(tc.tile_pool(name="nrmp", bufs=2))
            mp = ctx.enter_context(tc.tile_pool(name="mp", bufs=2))
            etp = ctx.enter_context(tc.tile_pool(name="etp", bufs=4))
            denp = ctx.enter_context(tc.tile_pool(name="denp", bufs=2))
            ytsp = ctx.enter_context(tc.tile_pool(name="ytsp", bufs=2))
            agp = ctx.enter_context(tc.tile_pool(name="agp", bufs=2))
            ysp = ctx.enter_context(tc.tile_pool(name="ysp", bufs=2))

            # PSUM: 2+2+2+2 = 8 banks
            proj_ps = ctx.enter_context(tc.tile_pool(name="proj_ps", bufs=2, space="PSUM"))
            s_ps = ctx.enter_context(tc.tile_pool(name="s_ps", bufs=2, space="PSUM"))
            yt_ps = ctx.enter_context(tc.tile_pool(name="yt_ps", bufs=2, space="PSUM"))
            bc_ps = ctx.enter_context(tc.tile_pool(name="bc_ps", bufs=2, space="PSUM"))

            # weight loads: wq per-ct on sync (interleaved with x chunk 0
            # below); wk/wv/wo as single rearranged DMAs on scalar
            nc.scalar.dma_start(
                out=wk_sb[:], in_=wk_in.ap().rearrange("(a p) f -> p a f", p=P))
            nc.scalar.dma_start(
                out=wv_sb[:], in_=wv_in.ap().rearrange("(a p) f -> p a f", p=P))
            nc.scalar.dma_start(
                out=wo_sb[:], in_=wo_in.ap().rearrange("(a p) f -> p a f", p=P))

            # tiny warm-up AllGather: pays the first-collective setup cost
            # and absorbs cross-core start skew off the critical path
            wu_d = dram.tile([1, 64], F16, name="wu_d")
            wu_o = dram.tile([4, 64], F16, name="wu_o")
            wu_sb = const.tile([1, 64], F16, name="wu_sb")
            nc.vector.memset(wu_sb[:], 0.0)
            nc.sync.dma_start(out=wu_d[:, :], in_=wu_sb[:])
            nc.gpsimd.collective_compute(
                "AllGather",
                mybir.AluOpType.bypass,
                replica_groups=[[0, 1, 2, 3], [4, 5, 6, 7]],
                ins=[wu_d[:].opt()],
                outs=[wu_o[:].opt()],
            )

            def emit_oproj(icc):
                q0, w = chunks[icc]
                ag_a = agp.tile([P, 8, 512], F16, name=f"ag_a{icc}", tag="ag")
                ag_b = agp.tile([P, 8, 512], F16, name=f"ag_b{icc}", tag="ag")
                for half, agt in ((0, ag_a), (1, ag_b)):
                    for m4 in range(2):
                        mt = half * 8 + m4 * 4
                        nc.sync.dma_start(
                            out=agt[:, m4 * 4:m4 * 4 + 4, 0:w],
                            in_=ag_ics[icc][mt * P:(mt + 4) * P, :]
                                .rearrange("(a p) f -> p a f", p=P),
                        )
                for oc in range(4):
                    yp = proj_ps.tile([P, w], F32, name=f"yp{icc}_{oc}", tag="proj")
                    for mt in range(CT):
                        agt = ag_a if mt < 8 else ag_b
                        nc.tensor.matmul(
                            yp[:],
                            wo_sb[:, mt, oc * P:(oc + 1) * P],
                            agt[:, mt % 8, 0:w],
                            start=(mt == 0), stop=(mt == CT - 1),
                        )
                    y_sb = ysp.tile([P, w], F32, name=f"ysb{icc}_{oc}", tag="ysb")
                    nc.scalar.copy(y_sb[:], yp[:])
                    nc.scalar.dma_start(
                        out=yt_out[oc * P:(oc + 1) * P, q0:q0 + w],
                        in_=y_sb[:],
                    )

            def emit_att(ci):
                q0, w = chunks[ci]
                njb = (q0 + w) // P
                jb0 = q0 // P
                for h in range(NHL):
                    ytp = yt_ps.tile([P, w], F32, name=f"yt{ci}_{h}", tag="yt")
                    den = bc_ps.tile([P, w], F32, name=f"den{ci}_{h}", tag="bc")
                    ets = [None] * njb
                    for jb in range(njb):
                        sp = s_ps.tile([P, w], F32, name=f"s{ci}_{h}_{jb}", tag="s")
                        nc.tensor.matmul(
                            sp[:],
                            kt_sb[:, h, jb * P:(jb + 1) * P],
                            qt_sb[:, h, q0:q0 + w],
                            start=True, stop=True,
                        )
                        et = etp.tile([P, w], F16, name=f"et{ci}_{h}_{jb}", tag="et")
                        nc.scalar.activation(
                            et[:], sp[:],
                            mybir.ActivationFunctionType.Exp,
                            bias=neg1_t[:], scale=SCALE,
                        )
                        t = jb - jb0
                        if t >= 0:
                            nc.vector.tensor_mul(et[:], et[:], mask_sb[:, t, 0:w])
                        ets[jb] = et
                        # AV and the ones-matmul denominator accumulation lag
                        # the score by one tile so PE never waits on exp
                        if jb >= 1:
                            nc.tensor.matmul(
                                ytp[:],
                                v_sb[:, jb - 1, h * HD:(h + 1) * HD],
                                ets[jb - 1][:],
                                start=(jb - 1 == 0), stop=False,
                            )
                            nc.tensor.matmul(
                                den[:], ones_h[:], ets[jb - 1][:],
                                start=(jb - 1 == 0), stop=False,
                            )
                    nc.tensor.matmul(
                        ytp[:],
                        v_sb[:, njb - 1, h * HD:(h + 1) * HD],
                        ets[njb - 1][:],
                        start=(njb == 1), stop=True,
                    )
                    nc.tensor.matmul(
                        den[:], ones_h[:], ets[njb - 1][:],
                        start=(njb == 1), stop=True,
                    )
                    # rden = exp(-ln(den)) on the scalar engine (same ACT
                    # table set as the softmax exp)
                    lnd = denp.tile([P, w], F32, name=f"lnd{ci}_{h}", tag="lnd", bufs=1)
                    nc.scalar.activation(
                        lnd[:], den[:], mybir.ActivationFunctionType.Ln)
                    rden = denp.tile([P, w], F32, name=f"rdn{ci}_{h}", tag="rden")
                    nc.scalar.activation(
                        rden[:], lnd[:], mybir.ActivationFunctionType.Exp,
                        scale=-1.0)
                    yt_sb = ytsp.tile([P, w], F16, name=f"yts{ci}_{h}", tag="yts")
                    nc.vector.tensor_mul(yt_sb[:], ytp[:], rden[:])
                    nc.scalar.dma_start(
                        out=yt_ics[ci][h * P:(h + 1) * P, :], in_=yt_sb[:])

                nc.gpsimd.collective_compute(
                    "AllGather",
                    mybir.AluOpType.bypass,
                    replica_groups=[[0, 1, 2, 3], [4, 5, 6, 7]],
                    ins=[yt_ics[ci][:].opt()],
                    outs=[ag_ics[ci][:].opt()],
                )

            for tch in range(NTCH):
                tc0 = tch * 512
                for half in range(2):
                    hc0 = tc0 + half * 256
                    xt_ch = xtp.tile(
                        [P, CT, 256], F16, name=f"xt{tch}_{half}", tag="xt")
                    for c4 in range(4):
                        ct = c4 * 4
                        if tch == 0 and half == 0:
                            nc.sync.dma_start(
                                out=wq_sb[:, ct:ct + 4, :],
                                in_=wq_in[ct * P:(ct + 4) * P, :]
                                    .rearrange("(a p) f -> p a f", p=P),
                            )
                        nc.sync.dma_start(
                            out=xt_ch[:, ct:ct + 4, :],
                            in_=xt_in[ct * P:(ct + 4) * P, hc0:hc0 + 256]
                                .rearrange("(a p) f -> p a f", p=P),
                        )

                    # ---- Q then K: transposed projection + rms-norm + rope ----
                    for wsb, dst, tag in ((wq_sb, qt_sb, "q"), (wk_sb, kt_sb, "k")):
                        raw4 = rawp.tile(
                            [P, NHL, 256], F16, name=f"{tag}raw{tch}_{half}", tag="raw")
                        nrm4 = nrmp.tile(
                            [P, NHL, 256], F16, name=f"{tag}nrm{tch}_{half}", tag="nrm")
                        sqs = []
                        for h in range(NHL):
                            ps = proj_ps.tile(
                                [P, 256], F32, name=f"{tag}ps{tch}_{half}_{h}",
                                tag="proj")
                            for ct in range(CT):
                                nc.tensor.matmul(
                                    ps[:],
                                    wsb[:, ct, h * P:(h + 1) * P],
                                    xt_ch[:, ct, :],
                                    start=(ct == 0), stop=(ct == CT - 1),
                                )
                            nc.vector.tensor_copy(raw4[:, h, :], ps[:])
                            sq = sqp.tile(
                                [P, 256], F16, name=f"{tag}sq{tch}_{half}_{h}",
                                tag="sq")
                            nc.vector.tensor_mul(sq[:], raw4[:, h, :], raw4[:, h, :])
                            sqs.append(sq)
                        for h in range(NHL):
                            ssum = bc_ps.tile(
                                [P, 256], F32, name=f"{tag}ss{tch}_{half}_{h}",
                                tag="bc")
                            nc.tensor.matmul(
                                ssum[:], ones_h[:], sqs[h][:], start=True, stop=True)
                            # rstd = exp(-0.5*ln(ms+eps)) — Ln and Exp share
                            # one ACT table set, so no table switches
                            lnt = sqp.tile(
                                [P, 256], F16, name=f"{tag}ln{tch}_{half}_{h}",
                                tag="lnt")
                            nc.scalar.activation(
                                lnt[:], ssum[:],
                                mybir.ActivationFunctionType.Ln,
                                bias=eps_t[:], scale=1.0 / HD,
                            )
                            nc.scalar.activation(
                                nrm4[:, h, :], lnt[:],
                                mybir.ActivationFunctionType.Exp,
                                scale=-0.5,
                            )
                        # rope: m1 = raw*cos_dup; m2 = swap(raw)*sin_signed
                        # (swap = partition rotation by 64 via PE permutation
                        # matmul); dst = (m1 + m2) * rstd
                        cosB = cs_sb[:, 0:1, hc0:hc0 + 256].broadcast_to((P, 2, 256))
                        sinB = cs_sb[:, 1:2, hc0:hc0 + 256].broadcast_to((P, 2, 256))
                        for g in range(2):
                            hs = slice(2 * g, 2 * g + 2)
                            qs_ps = s_ps.tile(
                                [P, 512], F32, name=f"{tag}qsw{tch}_{half}_{g}",
                                tag="s")
                            nc.tensor.matmul(
                                qs_ps[:], psw_sb[:],
                                raw4[:, hs, :], start=True, stop=True)
                            qsv = qs_ps[:].rearrange("p (h f) -> p h f", h=2)
                            m1 = mp.tile(
                                [P, 2, 256], F16,
                                name=f"{tag}m1{tch}_{half}_{g}", tag="m1")
                            m2 = mp.tile(
                                [P, 2, 256], F16,
                                name=f"{tag}m2{tch}_{half}_{g}", tag="m2")
                            nc.vector.tensor_mul(m1[:], raw4[:, hs, :], cosB)
                            nc.vector.tensor_mul(m2[:], qsv, sinB)
                            nc.vector.tensor_add(m1[:], m1[:], m2[:])
                            nc.vector.tensor_mul(
                                dst[:, hs, hc0:hc0 + 256], m1[:], nrm4[:, hs, :])

                    # ---- V: row-layout projection ----
                    for ib in range(2):
                        jb = tch * 4 + half * 2 + ib
                        ps = proj_ps.tile([P, GW], F32, name=f"vps{jb}", tag="proj")
                        for ct in range(CT):
                            nc.tensor.matmul(
                                ps[:],
                                xt_ch[:, ct, ib * P:(ib + 1) * P],
                                wv_sb[:, ct, :],
                                start=(ct == 0), stop=(ct == CT - 1),
                            )
                        nc.vector.tensor_copy(v_sb[:, jb, :], ps[:])

                    if tch == 3:
                        emit_att(3 if half == 0 else 4)

                if tch < 3:
                    emit_att(tch)
                if tch == 2:
                    emit_oproj(0)

            emit_oproj(1)
            emit_oproj(2)
            emit_oproj(3)
            emit_oproj(4)

    nc.compile()
    _program_cache["nc"] = nc
    return nc


def _rope_tables():
    inv_freq = 1.0 / (10000.0 ** (np.arange(0, HD, 2, dtype=np.float32) / HD))
    pos = np.arange(S, dtype=np.float32)
    freqs = np.outer(pos, inv_freq).astype(np.float32)  # [S, 64]
    cosT = np.cos(freqs).T  # [64, S]
    sinT = np.sin(freqs).T
    cs = np.empty((P, 2, S), dtype=np.float16)
    cs[0:64, 0] = cosT
    cs[64:128, 0] = cosT
    cs[0:64, 1] = sinT
    cs[64:128, 1] = -sinT
    return cs


def _mask_tiles():
    m = np.zeros((4, P, 512), dtype=np.float16)
    jj = np.arange(P)[:, None]
    ii = np.arange(512)[None, :]
    for t in range(4):
        m[t] = np.where(t * P + jj > ii, 0.0, 1.0)
    return m


def make_in_maps(x, Wq, Wk, Wv, Wo):
    x = np.asarray(x, dtype=np.float32)
    cs = _rope_tables()
    maskt = _mask_tiles()
    wqT = np.ascontiguousarray(np.asarray(Wq, dtype=np.float32).T.astype(np.float16))
    wkT = np.ascontiguousarray(np.asarray(Wk, dtype=np.float32).T.astype(np.float16))
    wvT = np.ascontiguousarray(np.asarray(Wv, dtype=np.float32).T.astype(np.float16))
    woT = np.ascontiguousarray(np.asarray(Wo, dtype=np.float32).T.astype(np.float16))
    xts = [np.ascontiguousarray(x[b].T.astype(np.float16)) for b in range(2)]
    psw = np.zeros((P, P), dtype=np.float16)
    kk = np.arange(P)
    psw[(kk + 64) % P, kk] = 1.0
    in_maps = []
    for c in range(NCORES):
        b, g = c // 4, c % 4
        sl = slice(g * GW, (g + 1) * GW)
        in_maps.append({
            "xt": xts[b],
            "wq": np.ascontiguousarray(wqT[:, sl]),
            "wk": np.ascontiguousarray(wkT[:, sl]),
            "wv": np.ascontiguousarray(wvT[:, sl]),
            "wo": np.ascontiguousarray(woT[:, sl]),
            "cs": cs,
            "maskt": maskt,
            "psw": psw,
        })
    return in_maps


def assemble_output(results):
    y = np.empty((2, S, D), dtype=np.float32)
    for c in range(NCORES):
        b, g = c // 4, c % 4
        y[b][:, g * GW:(g + 1) * GW] = results[c]["yt_out"].T
    return y


def kernel(x, Wq, Wk, Wv, Wo):
    nc = build_program()
    in_maps = make_in_maps(x, Wq, Wk, Wv, Wo)
    res = run_bass_kernel_spmd(nc, in_maps, core_ids=list(range(NCORES)))
    return assemble_output(res.results)


# revision 28
# speedup vs baseline: 1.1000x; 1.0068x over previous
"""Causal self-attention (QK-RMSNorm + RoPE) on 8 Trainium2 NeuronCores.

Problem: x[2,2048,2048], Wq/Wk/Wv/Wo [2048,2048], 16 heads, head_dim 128.

Sharding: core c handles batch b=c//4 and head group g=c%4 (4 heads,
model cols [512g:512g+512)).

Single fused pipeline, one pass over x per core:
- Q/K are projected directly into transposed [head_dim, tokens] layout
  by making the weight tile the stationary matmul operand (no PE
  transposes, no DRAM roundtrip).  V is projected in [tokens, cols]
  layout for the AV matmul.
- RMS-norm uses a ones[128,128] matmul to produce the per-token sum of
  squares broadcast across all partitions in one shot; normalization is
  a single DVE divide.  RoPE runs on 64-partition halves against a
  transposed cos/sin table.
- Attention per 512-token chunk uses transposed scores
  (eT = exp(scale*kT.T@qT - 1)); the softmax denominator is accumulated
  on the vector engine (csum += eT) and turned into a broadcast
  denominator with one ones-matmul per (head, chunk); yt = yt_acc / den.
- Per-chunk AllGather over the 4 cores of each batch (not all 8), then
  each core computes a 512-row slice of yT = Wo @ yt_full, interleaved
  at two chunks of lag so collectives hide under compute.
"""

import math
from contextlib import ExitStack

import numpy as np

import bass_rust as _bass_rust

import concourse.bass as bass
import concourse.bacc as bacc
import concourse.tile as tile
from concourse import mybir
from concourse.bass_utils import run_bass_kernel_spmd
from concourse.hw_specs import get_activation_tables

P = 128
D = 2048
S = 2048
HD = 128              # head dim
NHL = 4               # heads per core
GW = NHL * HD         # 512, per-core width of head group
CT = D // P           # 16 contraction tiles
NTCH = 4              # token chunks of 512
NCORES = 8
F32 = mybir.dt.float32
F16 = mybir.dt.float16
F32R = mybir.dt.float32r
SCALE = 1.0 / math.sqrt(HD)
EPS = 1.1920928955078125e-07

_program_cache = {}

# All scalar-engine transcendentals here are exp/ln/copy; route every one of
# them to the single ACT table set that contains them all so the table is
# loaded exactly once (the default chooser picks the first covering set per
# function, which alternates sets and costs ~2.7us per switch).
_SET_WITH_ALL = "natural_log_exp_and_others"
_SHARED_FNS = {
    mybir.ActivationFunctionType.Exp,
    mybir.ActivationFunctionType.Ln,
    mybir.ActivationFunctionType.Copy,
}


class _Bacc(bacc.Bacc):
    def insert_act_table_loads(self):
        has_activation = any(
            isinstance(i, mybir.InstActivation)
            for b in self.main_func.blocks
            for i in b.instructions
        )
        if not has_activation:
            return
        tables = []
        for name, fns in get_activation_tables(self.m.arch).items():
            if name != _SET_WITH_ALL:
                fns = fns - _SHARED_FNS
            tables.append((name, fns))
        _bass_rust.insert_act_table_loads(self, tables)


def build_program():
    if "nc" in _program_cache:
        return _program_cache["nc"]

    nc = _Bacc("TRN2", target_bir_lowering=False, debug=False, num_devices=NCORES)

    xt_in = nc.dram_tensor("xt", [D, S], F16, kind="ExternalInput")
    wq_in = nc.dram_tensor("wq", [D, GW], F16, kind="ExternalInput")
    wk_in = nc.dram_tensor("wk", [D, GW], F16, kind="ExternalInput")
    wv_in = nc.dram_tensor("wv", [D, GW], F16, kind="ExternalInput")
    wo_in = nc.dram_tensor("wo", [D, GW], F16, kind="ExternalInput")
    cs_in = nc.dram_tensor("cs", [P, 2, S], F16, kind="ExternalInput")
    mask_in = nc.dram_tensor("maskt", [4, P, 512], F16, kind="ExternalInput")
    psw_in = nc.dram_tensor("psw", [P, P], F16, kind="ExternalInput")
    yt_out = nc.dram_tensor("yt_out", [GW, S], F32, kind="ExternalOutput")

    with tile.TileContext(nc) as tc:
        with ExitStack() as ctx:
            const = ctx.enter_context(tc.tile_pool(name="const", bufs=1))
            dram = ctx.enter_context(tc.tile_pool(name="dram", bufs=1, space="DRAM"))

            eps_t = const.tile([P, 1], F32, name="eps_t")
            nc.vector.memset(eps_t[:], EPS)
            neg1_t = const.tile([P, 1], F32, name="neg1_t")
            nc.vector.memset(neg1_t[:], -1.0)
            ones_h = const.tile([P, P], F16, name="ones_h")
            nc.vector.memset(ones_h[:], 1.0)

            # plane 0: cos duplicated on both partition halves; plane 1:
            # +sin on rows 0..63, -sin on rows 64..127 (rope sign folded)
            cs_sb = const.tile([P, 2, S], F16, name="cs_sb")
            nc.sync.dma_start(out=cs_sb[:], in_=cs_in[:, :, :])
            mask_sb = const.tile([P, 4, 512], F16, name="mask_sb")
            nc.sync.dma_start(out=mask_sb[:], in_=mask_in.ap().rearrange("t p f -> p t f"))
            psw_sb = const.tile([P, P], F16, name="psw_sb")
            nc.scalar.dma_start(out=psw_sb[:], in_=psw_in[:, :])

            # attention chunks (q0, width)
            chunks = [(0, 512), (512, 512), (1024, 512), (1536, 512)]
            yt_ics = [
                dram.tile([GW, w], F16, name=f"yt_ic{i}")
                for i, (q0, w) in enumerate(chunks)
            ]
            ag_ics = [
                dram.tile([4 * GW, w], F16, name=f"ag_ic{i}")
                for i, (q0, w) in enumerate(chunks)
            ]

            # persistent SBUF
            wpool = ctx.enter_context(tc.tile_pool(name="wpool", bufs=1))
            wq_sb = wpool.tile([P, CT, GW], F16, name="wq_sb")
            wk_sb = wpool.tile([P, CT, GW], F16, name="wk_sb")
            wv_sb = wpool.tile([P, CT, GW], F16, name="wv_sb")
            wo_sb = wpool.tile([P, CT, GW], F16, name="wo_sb")
            qkv = ctx.enter_context(tc.tile_pool(name="qkv", bufs=1))
            qt_sb = qkv.tile([P, NHL, S], F16, name="qt_sb")
            kt_sb = qkv.tile([P, NHL, S], F16, name="kt_sb")
            v_sb = qkv.tile([P, CT, GW], F16, name="v_sb")

            # streaming pools (x in 256-token half-chunks)
            xtp = ctx.enter_context(tc.tile_pool(name="xtp", bufs=2))
            rawp = ctx.enter_context(tc.tile_pool(name="rawp", bufs=2))
            sqp = ctx.enter_context(tc.tile_pool(name="sqp", bufs=2))
            nrmp = ctx.enter_context(tc.tile_pool(name="nrmp", bufs=1))
            mp = ctx.enter_context(tc.tile_pool(name="mp", bufs=2))
            etp = ctx.enter_context(tc.tile_pool(name="etp", bufs=4))
            denp = ctx.enter_context(tc.tile_pool(name="denp", bufs=1))
            ytsp = ctx.enter_context(tc.tile_pool(name="ytsp", bufs=2))
            agp = ctx.enter_context(tc.tile_pool(name="agp", bufs=2))
            ysp = ctx.enter_context(tc.tile_pool(name="ysp", bufs=2))

            # PSUM: 2+2+2+2 = 8 banks
            proj_ps = ctx.enter_context(tc.tile_pool(name="proj_ps", bufs=2, space="PSUM"))
            s_ps = ctx.enter_context(tc.tile_pool(name="s_ps", bufs=2, space="PSUM"))
            yt_ps = ctx.enter_context(tc.tile_pool(name="yt_ps", bufs=2, space="PSUM"))
            bc_ps = ctx.enter_context(tc.tile_pool(name="bc_ps", bufs=2, space="PSUM"))

            # weight loads: wq per-ct on sync (interleaved with x chunk 0
            # below); wk/wv/wo as single rearranged DMAs on scalar
            nc.scalar.dma_start(
                out=wk_sb[:], in_=wk_in.ap().rearrange("(a p) f -> p a f", p=P))
            nc.scalar.dma_start(
                out=wv_sb[:], in_=wv_in.ap().rearrange("(a p) f -> p a f", p=P))
            nc.scalar.dma_start(
                out=wo_sb[:], in_=wo_in.ap().rearrange("(a p) f -> p a f", p=P))

            # tiny warm-up AllGather: pays the first-collective setup cost
            # and absorbs cross-core start skew off the critical path
            wu_d = dram.tile([1, 64], F16, name="wu_d")
            wu_o = dram.tile([4, 64], F16, name="wu_o")
            wu_sb = const.tile([1, 64], F16, name="wu_sb")
            nc.vector.memset(wu_sb[:], 0.0)
            nc.sync.dma_start(out=wu_d[:, :], in_=wu_sb[:])
            nc.gpsimd.collective_compute(
                "AllGather",
                mybir.AluOpType.bypass,
                replica_groups=[[0, 1, 2, 3], [4, 5, 6, 7]],
                ins=[wu_d[:].opt()],
                outs=[wu_o[:].opt()],
            )

            def emit_oproj(icc):
                q0, w = chunks[icc]
                ag_a = agp.tile([P, 8, 512], F16, name=f"ag_a{icc}", tag="ag")
                ag_b = agp.tile([P, 8, 512], F16, name=f"ag_b{icc}", tag="ag")
                for half, agt in ((0, ag_a), (1, ag_b)):
                    for m4 in range(2):
                        mt = half * 8 + m4 * 4
                        nc.sync.dma_start(
                            out=agt[:, m4 * 4:m4 * 4 + 4, 0:w],
                            in_=ag_ics[icc][mt * P:(mt + 4) * P, :]
                                .rearrange("(a p) f -> p a f", p=P),
                        )
                for oc in range(4):
                    yp = proj_ps.tile([P, w], F32, name=f"yp{icc}_{oc}", tag="proj")
                    for mt in range(CT):
                        agt = ag_a if mt < 8 else ag_b
                        nc.tensor.matmul(
                            yp[:],
                            wo_sb[:, mt, oc * P:(oc + 1) * P],
                            agt[:, mt % 8, 0:w],
                            start=(mt == 0), stop=(mt == CT - 1),
                        )
                    y_sb = ysp.tile([P, w], F32, name=f"ysb{icc}_{oc}", tag="ysb")
                    nc.scalar.copy(y_sb[:], yp[:])
                    nc.scalar.dma_start(
                        out=yt_out[oc * P:(oc + 1) * P, q0:q0 + w],
                        in_=y_sb[:],
                    )

            def emit_att(ci):
                q0, w = chunks[ci]
                njb = (q0 + w) // P
                jb0 = q0 // P
                for h in range(NHL):
                    ytp = yt_ps.tile([P, w], F32, name=f"yt{ci}_{h}", tag="yt")
                    den = bc_ps.tile([P, w], F32, name=f"den{ci}_{h}", tag="bc")
                    ets = [None] * njb
                    for jb in range(njb):
                        sp = s_ps.tile([P, w], F32, name=f"s{ci}_{h}_{jb}", tag="s")
                        nc.tensor.matmul(
                            sp[:],
                            kt_sb[:, h, jb * P:(jb + 1) * P],
                            qt_sb[:, h, q0:q0 + w],
                            start=True, stop=True,
                        )
                        et = etp.tile([P, w], F16, name=f"et{ci}_{h}_{jb}", tag="et")
                        nc.scalar.activation(
                            et[:], sp[:],
                            mybir.ActivationFunctionType.Exp,
                            bias=neg1_t[:], scale=SCALE,
                        )
                        t = jb - jb0
                        if t >= 0:
                            nc.vector.tensor_mul(et[:], et[:], mask_sb[:, t, 0:w])
                        ets[jb] = et
                        # AV and the ones-matmul denominator accumulation lag
                        # the score by one tile so PE never waits on exp
                        if jb >= 1:
                            nc.tensor.matmul(
                                ytp[:],
                                v_sb[:, jb - 1, h * HD:(h + 1) * HD],
                                ets[jb - 1][:],
                                start=(jb - 1 == 0), stop=False,
                            )
                            nc.tensor.matmul(
                                den[:], ones_h[:], ets[jb - 1][:],
                                start=(jb - 1 == 0), stop=False,
                            )
                    nc.tensor.matmul(
                        ytp[:],
                        v_sb[:, njb - 1, h * HD:(h + 1) * HD],
                        ets[njb - 1][:],
                        start=(njb == 1), stop=True,
                    )
                    nc.tensor.matmul(
                        den[:], ones_h[:], ets[njb - 1][:],
                        start=(njb == 1), stop=True,
                    )
                    # rden = exp(-ln(den)) on the scalar engine (same ACT
                    # table set as the softmax exp)
                    lnd = denp.tile([P, w], F32, name=f"lnd{ci}_{h}", tag="lnd", bufs=1)
                    nc.scalar.activation(
                        lnd[:], den[:], mybir.ActivationFunctionType.Ln)
                    rden = denp.tile([P, w], F32, name=f"rdn{ci}_{h}", tag="rden")
                    nc.scalar.activation(
                        rden[:], lnd[:], mybir.ActivationFunctionType.Exp,
                        scale=-1.0)
                    yt_sb = ytsp.tile([P, w], F16, name=f"yts{ci}_{h}", tag="yts")
                    nc.vector.tensor_mul(yt_sb[:], ytp[:], rden[:])
                    nc.scalar.dma_start(
                        out=yt_ics[ci][h * P:(h + 1) * P, :], in_=yt_sb[:])

                nc.gpsimd.collective_compute(
                    "AllGather",
                    mybir.AluOpType.bypass,
                    replica_groups=[[0, 1, 2, 3], [4, 5, 6, 7]],
                    ins=[yt_ics[ci][:].opt()],
                    outs=[ag_ics[ci][:].opt()],
                )

            for tch in range(NTCH):
                tc0 = tch * 512
                xt_ch = xtp.tile([P, CT, 512], F16, name=f"xt{tch}", tag="xt")
                for c4 in range(4):
                    ct = c4 * 4
                    if tch == 0:
                        nc.sync.dma_start(
                            out=wq_sb[:, ct:ct + 4, :],
                            in_=wq_in[ct * P:(ct + 4) * P, :]
                                .rearrange("(a p) f -> p a f", p=P),
                        )
                    nc.sync.dma_start(
                        out=xt_ch[:, ct:ct + 4, :],
                        in_=xt_in[ct * P:(ct + 4) * P, tc0:tc0 + 512]
                            .rearrange("(a p) f -> p a f", p=P),
                    )

                # ---- Q then K: transposed projection + rms-norm + rope ----
                for wsb, dst, tag in ((wq_sb, qt_sb, "q"), (wk_sb, kt_sb, "k")):
                    raw4 = rawp.tile(
                        [P, NHL, 512], F16, name=f"{tag}raw{tch}", tag="raw")
                    nrm4 = nrmp.tile(
                        [P, NHL, 512], F16, name=f"{tag}nrm{tch}", tag="nrm")
                    sqs = []
                    for h in range(NHL):
                        ps = proj_ps.tile(
                            [P, 512], F32, name=f"{tag}ps{tch}_{h}", tag="proj")
                        for ct in range(CT):
                            nc.tensor.matmul(
                                ps[:],
                                wsb[:, ct, h * P:(h + 1) * P],
                                xt_ch[:, ct, :],
                                start=(ct == 0), stop=(ct == CT - 1),
                            )
                        nc.vector.tensor_copy(raw4[:, h, :], ps[:])
                        sq = sqp.tile(
                            [P, 512], F16, name=f"{tag}sq{tch}_{h}", tag="sq")
                        nc.vector.tensor_mul(sq[:], raw4[:, h, :], raw4[:, h, :])
                        sqs.append(sq)
                    for h in range(NHL):
                        ssum = bc_ps.tile(
                            [P, 512], F32, name=f"{tag}ss{tch}_{h}", tag="bc")
                        nc.tensor.matmul(
                            ssum[:], ones_h[:], sqs[h][:], start=True, stop=True)
                        # rstd = exp(-0.5*ln(ms+eps)) — Ln and Exp share
                        # one ACT table set, so no table switches
                        lnt = sqp.tile(
                            [P, 512], F16, name=f"{tag}ln{tch}_{h}", tag="lnt")
                        nc.scalar.activation(
                            lnt[:], ssum[:],
                            mybir.ActivationFunctionType.Ln,
                            bias=eps_t[:], scale=1.0 / HD,
                        )
                        nc.scalar.activation(
                            nrm4[:, h, :], lnt[:],
                            mybir.ActivationFunctionType.Exp,
                            scale=-0.5,
                        )
                    # rope: m1 = raw*cos_dup; m2 = swap(raw)*sin_signed
                    # (swap = partition rotation by 64 via PE permutation
                    # matmul); dst = (m1 + m2) * rstd
                    cosB = cs_sb[:, 0:1, tc0:tc0 + 512].broadcast_to((P, 1, 512))
                    sinB = cs_sb[:, 1:2, tc0:tc0 + 512].broadcast_to((P, 1, 512))
                    for g in range(NHL):
                        hs = slice(g, g + 1)
                        qs_ps = s_ps.tile(
                            [P, 512], F32, name=f"{tag}qsw{tch}_{g}", tag="s")
                        nc.tensor.matmul(
                            qs_ps[:], psw_sb[:],
                            raw4[:, g, :], start=True, stop=True)
                        qsv = qs_ps[:].rearrange("p (h f) -> p h f", h=1)
                        m1 = mp.tile(
                            [P, 1, 512], F16, name=f"{tag}m1{tch}_{g}", tag="m1")
                        m2 = mp.tile(
                            [P, 1, 512], F16, name=f"{tag}m2{tch}_{g}", tag="m2")
                        nc.vector.tensor_mul(m1[:], raw4[:, hs, :], cosB)
                        nc.vector.tensor_mul(m2[:], qsv, sinB)
                        nc.vector.tensor_add(m1[:], m1[:], m2[:])
                        nc.vector.tensor_mul(
                            dst[:, hs, tc0:tc0 + 512], m1[:], nrm4[:, hs, :])

                # ---- V: row-layout projection ----
                for ib in range(4):
                    jb = tch * 4 + ib
                    ps = proj_ps.tile([P, GW], F32, name=f"vps{jb}", tag="proj")
                    for ct in range(CT):
                        nc.tensor.matmul(
                            ps[:],
                            xt_ch[:, ct, ib * P:(ib + 1) * P],
                            wv_sb[:, ct, :],
                            start=(ct == 0), stop=(ct == CT - 1),
                        )
                    nc.vector.tensor_copy(v_sb[:, jb, :], ps[:])

                emit_att(tch)
                if tch == 2:
                    emit_oproj(0)

            emit_oproj(1)
            emit_oproj(2)
            emit_oproj(3)

    nc.compile()
    _program_cache["nc"] = nc
    return nc


def _rope_tables():
    inv_freq = 1.0 / (10000.0 ** (np.arange(0, HD, 2, dtype=np.float32) / HD))
    pos = np.arange(S, dtype=np.float32)
    freqs = np.outer(pos, inv_freq).astype(np.float32)  # [S, 64]
    cosT = np.cos(freqs).T  # [64, S]
    sinT = np.sin(freqs).T
    cs = np.empty((P, 2, S), dtype=np.float16)
    cs[0:64, 0] = cosT
    cs[64:128, 0] = cosT
    cs[0:64, 1] = sinT
    cs[64:128, 1] = -sinT
    return cs


def _mask_tiles():
    m = np.zeros((4, P, 512), dtype=np.float16)
    jj = np.arange(P)[:, None]
    ii = np.arange(512)[None, :]
    for t in range(4):
        m[t] = np.where(t * P + jj > ii, 0.0, 1.0)
    return m


def make_in_maps(x, Wq, Wk, Wv, Wo):
    x = np.asarray(x, dtype=np.float32)
    cs = _rope_tables()
    maskt = _mask_tiles()
    wqT = np.ascontiguousarray(np.asarray(Wq, dtype=np.float32).T.astype(np.float16))
    wkT = np.ascontiguousarray(np.asarray(Wk, dtype=np.float32).T.astype(np.float16))
    wvT = np.ascontiguousarray(np.asarray(Wv, dtype=np.float32).T.astype(np.float16))
    woT = np.ascontiguousarray(np.asarray(Wo, dtype=np.float32).T.astype(np.float16))
    xts = [np.ascontiguousarray(x[b].T.astype(np.float16)) for b in range(2)]
    psw = np.zeros((P, P), dtype=np.float16)
    kk = np.arange(P)
    psw[(kk + 64) % P, kk] = 1.0
    in_maps = []
    for c in range(NCORES):
        b, g = c // 4, c % 4
        sl = slice(g * GW, (g + 1) * GW)
        in_maps.append({
            "xt": xts[b],
            "wq": np.ascontiguousarray(wqT[:, sl]),
            "wk": np.ascontiguousarray(wkT[:, sl]),
            "wv": np.ascontiguousarray(wvT[:, sl]),
            "wo": np.ascontiguousarray(woT[:, sl]),
            "cs": cs,
            "maskt": maskt,
            "psw": psw,
        })
    return in_maps


def assemble_output(results):
    y = np.empty((2, S, D), dtype=np.float32)
    for c in range(NCORES):
        b, g = c // 4, c % 4
        y[b][:, g * GW:(g + 1) * GW] = results[c]["yt_out"].T
    return y


def kernel(x, Wq, Wk, Wv, Wo):
    nc = build_program()
    in_maps = make_in_maps(x, Wq, Wk, Wv, Wo)
    res = run_bass_kernel_spmd(nc, in_maps, core_ids=list(range(NCORES)))
    return assemble_output(res.results)


# revision 31
# speedup vs baseline: 1.1716x; 1.0651x over previous
"""Causal self-attention (QK-RMSNorm + RoPE) on 8 Trainium2 NeuronCores.

Problem: x[2,2048,2048], Wq/Wk/Wv/Wo [2048,2048], 16 heads, head_dim 128.

Sharding: core c handles batch b=c//4 and head group g=c%4 (4 heads,
model cols [512g:512g+512)).

Single fused pipeline, one pass over x per core:
- Q/K are projected directly into transposed [head_dim, tokens] layout
  by making the weight tile the stationary matmul operand (no PE
  transposes, no DRAM roundtrip).  V is projected in [tokens, cols]
  layout for the AV matmul.
- RMS-norm uses a ones[128,128] matmul to produce the per-token sum of
  squares broadcast across all partitions in one shot; normalization is
  a single DVE divide.  RoPE runs on 64-partition halves against a
  transposed cos/sin table.
- Attention per 512-token chunk uses transposed scores
  (eT = exp(scale*kT.T@qT - 1)); the softmax denominator is accumulated
  on the vector engine (csum += eT) and turned into a broadcast
  denominator with one ones-matmul per (head, chunk); yt = yt_acc / den.
- Per-chunk AllGather over the 4 cores of each batch (not all 8), then
  each core computes a 512-row slice of yT = Wo @ yt_full, interleaved
  at two chunks of lag so collectives hide under compute.
"""

import math
from contextlib import ExitStack

import numpy as np

import bass_rust as _bass_rust

import concourse.bass as bass
import concourse.bacc as bacc
import concourse.tile as tile
from concourse import mybir
from concourse.bass_utils import run_bass_kernel_spmd
from concourse.hw_specs import get_activation_tables

P = 128
D = 2048
S = 2048
HD = 128              # head dim
NHL = 4               # heads per core
GW = NHL * HD         # 512, per-core width of head group
CT = D // P           # 16 contraction tiles
NTCH = 4              # token chunks of 512
NCORES = 8
F32 = mybir.dt.float32
F16 = mybir.dt.float16
F32R = mybir.dt.float32r
SCALE = 1.0 / math.sqrt(HD)
EPS = 1.1920928955078125e-07

_program_cache = {}

# All scalar-engine transcendentals here are exp/ln/copy; route every one of
# them to the single ACT table set that contains them all so the table is
# loaded exactly once (the default chooser picks the first covering set per
# function, which alternates sets and costs ~2.7us per switch).
_SET_WITH_ALL = "natural_log_exp_and_others"
_SHARED_FNS = {
    mybir.ActivationFunctionType.Exp,
    mybir.ActivationFunctionType.Ln,
    mybir.ActivationFunctionType.Copy,
}


class _Bacc(bacc.Bacc):
    def insert_act_table_loads(self):
        has_activation = any(
            isinstance(i, mybir.InstActivation)
            for b in self.main_func.blocks
            for i in b.instructions
        )
        if not has_activation:
            return
        tables = []
        for name, fns in get_activation_tables(self.m.arch).items():
            if name != _SET_WITH_ALL:
                fns = fns - _SHARED_FNS
            tables.append((name, fns))
        _bass_rust.insert_act_table_loads(self, tables)


def build_program():
    if "nc" in _program_cache:
        return _program_cache["nc"]

    nc = _Bacc("TRN2", target_bir_lowering=False, debug=False, num_devices=NCORES)

    xt_in = nc.dram_tensor("xt", [D, S], F16, kind="ExternalInput")
    wq_in = nc.dram_tensor("wq", [D, GW], F16, kind="ExternalInput")
    wk_in = nc.dram_tensor("wk", [D, GW], F16, kind="ExternalInput")
    wv_in = nc.dram_tensor("wv", [D, GW], F16, kind="ExternalInput")
    wo_in = nc.dram_tensor("wo", [D, GW], F16, kind="ExternalInput")
    cs_in = nc.dram_tensor("cs", [P, 2, S], F16, kind="ExternalInput")
    mask_in = nc.dram_tensor("maskt", [4, P, 512], F16, kind="ExternalInput")
    psw_in = nc.dram_tensor("psw", [P, P], F16, kind="ExternalInput")
    yt_out = nc.dram_tensor("yt_out", [GW, S], F32, kind="ExternalOutput")

    with tile.TileContext(nc) as tc:
        with ExitStack() as ctx:
            const = ctx.enter_context(tc.tile_pool(name="const", bufs=1))
            dram = ctx.enter_context(tc.tile_pool(name="dram", bufs=1, space="DRAM"))

            eps_t = const.tile([P, 1], F32, name="eps_t")
            nc.vector.memset(eps_t[:], EPS)
            neg1_t = const.tile([P, 1], F32, name="neg1_t")
            nc.vector.memset(neg1_t[:], -1.0)
            ones_h = const.tile([P, P], F16, name="ones_h")
            nc.vector.memset(ones_h[:], 1.0)
            ones_b = const.tile([P, P], mybir.dt.bfloat16, name="ones_b")
            nc.vector.memset(ones_b[:], 1.0)

            # plane 0: cos duplicated on both partition halves; plane 1:
            # +sin on rows 0..63, -sin on rows 64..127 (rope sign folded)
            cs_sb = const.tile([P, 2, S], F16, name="cs_sb")
            nc.sync.dma_start(out=cs_sb[:], in_=cs_in[:, :, :])
            mask_sb = const.tile([P, 4, 512], F16, name="mask_sb")
            nc.sync.dma_start(out=mask_sb[:], in_=mask_in.ap().rearrange("t p f -> p t f"))
            psw_sb = const.tile([P, P], F16, name="psw_sb")
            nc.scalar.dma_start(out=psw_sb[:], in_=psw_in[:, :])

            # attention chunks (q0, width)
            chunks = [(0, 512), (512, 512), (1024, 512), (1536, 512)]
            yt_ics = [
                dram.tile([GW, w], F16, name=f"yt_ic{i}")
                for i, (q0, w) in enumerate(chunks)
            ]
            ag_ics = [
                dram.tile([4 * GW, w], F16, name=f"ag_ic{i}")
                for i, (q0, w) in enumerate(chunks)
            ]

            # persistent SBUF
            wpool = ctx.enter_context(tc.tile_pool(name="wpool", bufs=1))
            wq_sb = wpool.tile([P, CT, GW], F16, name="wq_sb")
            wk_sb = wpool.tile([P, CT, GW], F16, name="wk_sb")
            wv_sb = wpool.tile([P, CT, GW], F16, name="wv_sb")
            wo_sb = wpool.tile([P, CT, GW], F16, name="wo_sb")
            qkv = ctx.enter_context(tc.tile_pool(name="qkv", bufs=1))
            qt_sb = qkv.tile([P, NHL, S], F16, name="qt_sb")
            kt_sb = qkv.tile([P, NHL, S], F16, name="kt_sb")
            v_sb = qkv.tile([P, CT, GW], F16, name="v_sb")

            # streaming pools (x in 256-token half-chunks)
            xtp = ctx.enter_context(tc.tile_pool(name="xtp", bufs=2))
            rawp = ctx.enter_context(tc.tile_pool(name="rawp", bufs=2))
            sqp = ctx.enter_context(tc.tile_pool(name="sqp", bufs=2))
            nrmp = ctx.enter_context(tc.tile_pool(name="nrmp", bufs=1))
            mp = ctx.enter_context(tc.tile_pool(name="mp", bufs=1))
            etp = ctx.enter_context(tc.tile_pool(name="etp", bufs=3))
            csp = ctx.enter_context(tc.tile_pool(name="csp", bufs=2))
            denp = ctx.enter_context(tc.tile_pool(name="denp", bufs=1))
            ytsp = ctx.enter_context(tc.tile_pool(name="ytsp", bufs=2))
            agp = ctx.enter_context(tc.tile_pool(name="agp", bufs=2))
            ysp = ctx.enter_context(tc.tile_pool(name="ysp", bufs=2))

            # PSUM: 2+2+2+2 = 8 banks
            proj_ps = ctx.enter_context(tc.tile_pool(name="proj_ps", bufs=2, space="PSUM"))
            s_ps = ctx.enter_context(tc.tile_pool(name="s_ps", bufs=2, space="PSUM"))
            yt_ps = ctx.enter_context(tc.tile_pool(name="yt_ps", bufs=2, space="PSUM"))
            bc_ps = ctx.enter_context(tc.tile_pool(name="bc_ps", bufs=2, space="PSUM"))

            # weight loads: wq per-ct on sync (interleaved with x chunk 0
            # below); wk/wv/wo as single rearranged DMAs on scalar
            nc.scalar.dma_start(
                out=wk_sb[:], in_=wk_in.ap().rearrange("(a p) f -> p a f", p=P))
            nc.scalar.dma_start(
                out=wv_sb[:], in_=wv_in.ap().rearrange("(a p) f -> p a f", p=P))
            nc.scalar.dma_start(
                out=wo_sb[:], in_=wo_in.ap().rearrange("(a p) f -> p a f", p=P))

            # tiny warm-up AllGather: pays the first-collective setup cost
            # and absorbs cross-core start skew off the critical path
            wu_d = dram.tile([1, 64], F16, name="wu_d")
            wu_o = dram.tile([4, 64], F16, name="wu_o")
            wu_sb = const.tile([1, 64], F16, name="wu_sb")
            nc.vector.memset(wu_sb[:], 0.0)
            nc.sync.dma_start(out=wu_d[:, :], in_=wu_sb[:])
            nc.gpsimd.collective_compute(
                "AllGather",
                mybir.AluOpType.bypass,
                replica_groups=[[0, 1, 2, 3], [4, 5, 6, 7]],
                ins=[wu_d[:].opt()],
                outs=[wu_o[:].opt()],
            )

            def emit_oproj(icc):
                q0, w = chunks[icc]
                ag_a = agp.tile([P, 8, 512], F16, name=f"ag_a{icc}", tag="ag")
                ag_b = agp.tile([P, 8, 512], F16, name=f"ag_b{icc}", tag="ag")
                for half, agt in ((0, ag_a), (1, ag_b)):
                    for m4 in range(2):
                        mt = half * 8 + m4 * 4
                        nc.sync.dma_start(
                            out=agt[:, m4 * 4:m4 * 4 + 4, 0:w],
                            in_=ag_ics[icc][mt * P:(mt + 4) * P, :]
                                .rearrange("(a p) f -> p a f", p=P),
                        )
                for oc in range(4):
                    yp = proj_ps.tile([P, w], F32, name=f"yp{icc}_{oc}", tag="proj")
                    for mt in range(CT):
                        agt = ag_a if mt < 8 else ag_b
                        nc.tensor.matmul(
                            yp[:],
                            wo_sb[:, mt, oc * P:(oc + 1) * P],
                            agt[:, mt % 8, 0:w],
                            start=(mt == 0), stop=(mt == CT - 1),
                        )
                    y_sb = ysp.tile([P, w], F32, name=f"ysb{icc}_{oc}", tag="ysb")
                    nc.scalar.copy(y_sb[:], yp[:])
                    nc.scalar.dma_start(
                        out=yt_out[oc * P:(oc + 1) * P, q0:q0 + w],
                        in_=y_sb[:],
                    )

            def emit_att(ci):
                q0, w = chunks[ci]
                njb = (q0 + w) // P
                jb0 = q0 // P
                for h in range(NHL):
                    ytp = yt_ps.tile([P, w], F32, name=f"yt{ci}_{h}", tag="yt")
                    csum = csp.tile(
                        [P, w], mybir.dt.bfloat16, name=f"cs{ci}_{h}", tag="cs")
                    ets = [None] * njb
                    for jb in range(njb):
                        sp = s_ps.tile([P, w], F32, name=f"s{ci}_{h}_{jb}", tag="s")
                        nc.tensor.matmul(
                            sp[:],
                            kt_sb[:, h, jb * P:(jb + 1) * P],
                            qt_sb[:, h, q0:q0 + w],
                            start=True, stop=True,
                        )
                        et = etp.tile([P, w], F16, name=f"et{ci}_{h}_{jb}", tag="et")
                        nc.scalar.activation(
                            et[:], sp[:],
                            mybir.ActivationFunctionType.Exp,
                            bias=neg1_t[:], scale=SCALE,
                        )
                        t = jb - jb0
                        if t >= 0:
                            nc.vector.tensor_mul(et[:], et[:], mask_sb[:, t, 0:w])
                        if jb == 0:
                            nc.vector.tensor_copy(csum[:], et[:])
                        else:
                            nc.vector.tensor_add(csum[:], csum[:], et[:])
                        ets[jb] = et
                        # AV lags the score by one tile so PE never waits
                        if jb >= 1:
                            nc.tensor.matmul(
                                ytp[:],
                                v_sb[:, jb - 1, h * HD:(h + 1) * HD],
                                ets[jb - 1][:],
                                start=(jb - 1 == 0), stop=False,
                            )
                    nc.tensor.matmul(
                        ytp[:],
                        v_sb[:, njb - 1, h * HD:(h + 1) * HD],
                        ets[njb - 1][:],
                        start=(njb == 1), stop=True,
                    )
                    den = bc_ps.tile([P, w], F32, name=f"den{ci}_{h}", tag="bc")
                    nc.tensor.matmul(
                        den[:], ones_b[:], csum[:], start=True, stop=True)
                    # rden = exp(-ln(den)) on the scalar engine (same ACT
                    # table set as the softmax exp)
                    lnd = denp.tile([P, w], F32, name=f"lnd{ci}_{h}", tag="lnd", bufs=1)
                    nc.scalar.activation(
                        lnd[:], den[:], mybir.ActivationFunctionType.Ln)
                    rden = denp.tile([P, w], F32, name=f"rdn{ci}_{h}", tag="rden")
                    nc.scalar.activation(
                        rden[:], lnd[:], mybir.ActivationFunctionType.Exp,
                        scale=-1.0)
                    yt_sb = ytsp.tile([P, w], F16, name=f"yts{ci}_{h}", tag="yts")
                    nc.vector.tensor_mul(yt_sb[:], ytp[:], rden[:])
                    nc.scalar.dma_start(
                        out=yt_ics[ci][h * P:(h + 1) * P, :], in_=yt_sb[:])

                nc.gpsimd.collective_compute(
                    "AllGather",
                    mybir.AluOpType.bypass,
                    replica_groups=[[0, 1, 2, 3], [4, 5, 6, 7]],
                    ins=[yt_ics[ci][:].opt()],
                    outs=[ag_ics[ci][:].opt()],
                )

            for tch in range(NTCH):
                tc0 = tch * 512
                xt_ch = xtp.tile([P, CT, 512], F16, name=f"xt{tch}", tag="xt")
                for c4 in range(4):
                    ct = c4 * 4
                    if tch == 0:
                        nc.sync.dma_start(
                            out=wq_sb[:, ct:ct + 4, :],
                            in_=wq_in[ct * P:(ct + 4) * P, :]
                                .rearrange("(a p) f -> p a f", p=P),
                        )
                    nc.sync.dma_start(
                        out=xt_ch[:, ct:ct + 4, :],
                        in_=xt_in[ct * P:(ct + 4) * P, tc0:tc0 + 512]
                            .rearrange("(a p) f -> p a f", p=P),
                    )

                # ---- Q then K: transposed projection + rms-norm + rope ----
                for wsb, dst, tag in ((wq_sb, qt_sb, "q"), (wk_sb, kt_sb, "k")):
                    raw4 = rawp.tile(
                        [P, NHL, 512], F16, name=f"{tag}raw{tch}", tag="raw")
                    nrm4 = nrmp.tile(
                        [P, NHL, 512], F16, name=f"{tag}nrm{tch}", tag="nrm")
                    sqs = []
                    for h in range(NHL):
                        ps = proj_ps.tile(
                            [P, 512], F32, name=f"{tag}ps{tch}_{h}", tag="proj")
                        for ct in range(CT):
                            nc.tensor.matmul(
                                ps[:],
                                wsb[:, ct, h * P:(h + 1) * P],
                                xt_ch[:, ct, :],
                                start=(ct == 0), stop=(ct == CT - 1),
                            )
                        nc.vector.tensor_copy(raw4[:, h, :], ps[:])
                        sq = sqp.tile(
                            [P, 512], F16, name=f"{tag}sq{tch}_{h}", tag="sq")
                        nc.vector.tensor_mul(sq[:], raw4[:, h, :], raw4[:, h, :])
                        sqs.append(sq)
                    for h in range(NHL):
                        ssum = bc_ps.tile(
                            [P, 512], F32, name=f"{tag}ss{tch}_{h}", tag="bc")
                        nc.tensor.matmul(
                            ssum[:], ones_h[:], sqs[h][:], start=True, stop=True)
                        # rstd = exp(-0.5*ln(ms+eps)) — Ln and Exp share
                        # one ACT table set, so no table switches
                        lnt = sqp.tile(
                            [P, 512], F16, name=f"{tag}ln{tch}_{h}", tag="lnt")
                        nc.scalar.activation(
                            lnt[:], ssum[:],
                            mybir.ActivationFunctionType.Ln,
                            bias=eps_t[:], scale=1.0 / HD,
                        )
                        nc.scalar.activation(
                            nrm4[:, h, :], lnt[:],
                            mybir.ActivationFunctionType.Exp,
                            scale=-0.5,
                        )
                    # rope: m1 = raw*cos_dup; m2 = swap(raw)*sin_signed
                    # (swap = partition rotation by 64 via PE permutation
                    # matmul); dst = (m1 + m2) * rstd
                    cosB = cs_sb[:, 0:1, tc0:tc0 + 512].broadcast_to((P, 1, 512))
                    sinB = cs_sb[:, 1:2, tc0:tc0 + 512].broadcast_to((P, 1, 512))
                    for g in range(NHL):
                        hs = slice(g, g + 1)
                        qs_ps = s_ps.tile(
                            [P, 512], F32, name=f"{tag}qsw{tch}_{g}", tag="s")
                        nc.tensor.matmul(
                            qs_ps[:], psw_sb[:],
                            raw4[:, g, :], start=True, stop=True)
                        qsv = qs_ps[:].rearrange("p (h f) -> p h f", h=1)
                        m1 = mp.tile(
                            [P, 1, 512], F16, name=f"{tag}m1{tch}_{g}", tag="m1")
                        m2 = mp.tile(
                            [P, 1, 512], F16, name=f"{tag}m2{tch}_{g}", tag="m2")
                        nc.vector.tensor_mul(m1[:], raw4[:, hs, :], cosB)
                        nc.vector.tensor_mul(m2[:], qsv, sinB)
                        nc.vector.tensor_add(m1[:], m1[:], m2[:])
                        nc.vector.tensor_mul(
                            dst[:, hs, tc0:tc0 + 512], m1[:], nrm4[:, hs, :])

                # ---- V: row-layout projection ----
                for ib in range(4):
                    jb = tch * 4 + ib
                    ps = proj_ps.tile([P, GW], F32, name=f"vps{jb}", tag="proj")
                    for ct in range(CT):
                        nc.tensor.matmul(
                            ps[:],
                            xt_ch[:, ct, ib * P:(ib + 1) * P],
                            wv_sb[:, ct, :],
                            start=(ct == 0), stop=(ct == CT - 1),
                        )
                    nc.vector.tensor_copy(v_sb[:, jb, :], ps[:])

                emit_att(tch)
                if tch == 2:
                    emit_oproj(0)

            emit_oproj(1)
            emit_oproj(2)
            emit_oproj(3)

    nc.compile()
    _program_cache["nc"] = nc
    return nc


def _rope_tables():
    inv_freq = 1.0 / (10000.0 ** (np.arange(0, HD, 2, dtype=np.float32) / HD))
    pos = np.arange(S, dtype=np.float32)
    freqs = np.outer(pos, inv_freq).astype(np.float32)  # [S, 64]
    cosT = np.cos(freqs).T  # [64, S]
    sinT = np.sin(freqs).T
    cs = np.empty((P, 2, S), dtype=np.float16)
    cs[0:64, 0] = cosT
    cs[64:128, 0] = cosT
    cs[0:64, 1] = sinT
    cs[64:128, 1] = -sinT
    return cs


def _mask_tiles():
    m = np.zeros((4, P, 512), dtype=np.float16)
    jj = np.arange(P)[:, None]
    ii = np.arange(512)[None, :]
    for t in range(4):
        m[t] = np.where(t * P + jj > ii, 0.0, 1.0)
    return m


def make_in_maps(x, Wq, Wk, Wv, Wo):
    x = np.asarray(x, dtype=np.float32)
    cs = _rope_tables()
    maskt = _mask_tiles()
    wqT = np.ascontiguousarray(np.asarray(Wq, dtype=np.float32).T.astype(np.float16))
    wkT = np.ascontiguousarray(np.asarray(Wk, dtype=np.float32).T.astype(np.float16))
    wvT = np.ascontiguousarray(np.asarray(Wv, dtype=np.float32).T.astype(np.float16))
    woT = np.ascontiguousarray(np.asarray(Wo, dtype=np.float32).T.astype(np.float16))
    xts = [np.ascontiguousarray(x[b].T.astype(np.float16)) for b in range(2)]
    psw = np.zeros((P, P), dtype=np.float16)
    kk = np.arange(P)
    psw[(kk + 64) % P, kk] = 1.0
    in_maps = []
    for c in range(NCORES):
        b, g = c // 4, c % 4
        sl = slice(g * GW, (g + 1) * GW)
        in_maps.append({
            "xt": xts[b],
            "wq": np.ascontiguousarray(wqT[:, sl]),
            "wk": np.ascontiguousarray(wkT[:, sl]),
            "wv": np.ascontiguousarray(wvT[:, sl]),
            "wo": np.ascontiguousarray(woT[:, sl]),
            "cs": cs,
            "maskt": maskt,
            "psw": psw,
        })
    return in_maps


def assemble_output(results):
    y = np.empty((2, S, D), dtype=np.float32)
    for c in range(NCORES):
        b, g = c // 4, c % 4
        y[b][:, g * GW:(g + 1) * GW] = results[c]["yt_out"].T
    return y


def kernel(x, Wq, Wk, Wv, Wo):
    nc = build_program()
    in_maps = make_in_maps(x, Wq, Wk, Wv, Wo)
    res = run_bass_kernel_spmd(nc, in_maps, core_ids=list(range(NCORES)))
    return assemble_output(res.results)


# revision 32
# speedup vs baseline: 1.2136x; 1.0359x over previous
"""Causal self-attention (QK-RMSNorm + RoPE) on 8 Trainium2 NeuronCores.

Problem: x[2,2048,2048], Wq/Wk/Wv/Wo [2048,2048], 16 heads, head_dim 128.

Sharding: core c handles batch b=c//4 and head group g=c%4 (4 heads,
model cols [512g:512g+512)).

Single fused pipeline, one pass over x per core:
- Q/K are projected directly into transposed [head_dim, tokens] layout
  by making the weight tile the stationary matmul operand (no PE
  transposes, no DRAM roundtrip).  V is projected in [tokens, cols]
  layout for the AV matmul.
- RMS-norm uses a ones[128,128] matmul to produce the per-token sum of
  squares broadcast across all partitions in one shot; normalization is
  a single DVE divide.  RoPE runs on 64-partition halves against a
  transposed cos/sin table.
- Attention per 512-token chunk uses transposed scores
  (eT = exp(scale*kT.T@qT - 1)); the softmax denominator is accumulated
  on the vector engine (csum += eT) and turned into a broadcast
  denominator with one ones-matmul per (head, chunk); yt = yt_acc / den.
- Per-chunk AllGather over the 4 cores of each batch (not all 8), then
  each core computes a 512-row slice of yT = Wo @ yt_full, interleaved
  at two chunks of lag so collectives hide under compute.
"""

import math
from contextlib import ExitStack

import numpy as np

import bass_rust as _bass_rust

import concourse.bass as bass
import concourse.bacc as bacc
import concourse.tile as tile
from concourse import mybir
from concourse.bass_utils import run_bass_kernel_spmd
from concourse.hw_specs import get_activation_tables

P = 128
D = 2048
S = 2048
HD = 128              # head dim
NHL = 4               # heads per core
GW = NHL * HD         # 512, per-core width of head group
CT = D // P           # 16 contraction tiles
NTCH = 4              # token chunks of 512
NCORES = 8
F32 = mybir.dt.float32
F16 = mybir.dt.float16
F32R = mybir.dt.float32r
SCALE = 1.0 / math.sqrt(HD)
EPS = 1.1920928955078125e-07

_program_cache = {}

# All scalar-engine transcendentals here are exp/ln/copy; route every one of
# them to the single ACT table set that contains them all so the table is
# loaded exactly once (the default chooser picks the first covering set per
# function, which alternates sets and costs ~2.7us per switch).
_SET_WITH_ALL = "natural_log_exp_and_others"
_SHARED_FNS = {
    mybir.ActivationFunctionType.Exp,
    mybir.ActivationFunctionType.Ln,
    mybir.ActivationFunctionType.Copy,
}


class _Bacc(bacc.Bacc):
    def insert_act_table_loads(self):
        has_activation = any(
            isinstance(i, mybir.InstActivation)
            for b in self.main_func.blocks
            for i in b.instructions
        )
        if not has_activation:
            return
        tables = []
        for name, fns in get_activation_tables(self.m.arch).items():
            if name != _SET_WITH_ALL:
                fns = fns - _SHARED_FNS
            tables.append((name, fns))
        _bass_rust.insert_act_table_loads(self, tables)


def build_program():
    if "nc" in _program_cache:
        return _program_cache["nc"]

    nc = _Bacc("TRN2", target_bir_lowering=False, debug=False, num_devices=NCORES)

    xt_in = nc.dram_tensor("xt", [D, S], F16, kind="ExternalInput")
    wq_in = nc.dram_tensor("wq", [D, GW], F16, kind="ExternalInput")
    wk_in = nc.dram_tensor("wk", [D, GW], F16, kind="ExternalInput")
    wv_in = nc.dram_tensor("wv", [D, GW], F16, kind="ExternalInput")
    wo_in = nc.dram_tensor("wo", [D, GW], F16, kind="ExternalInput")
    cs_in = nc.dram_tensor("cs", [P, 2, S], F16, kind="ExternalInput")
    mask_in = nc.dram_tensor("maskt", [4, P, 512], F16, kind="ExternalInput")
    psw_in = nc.dram_tensor("psw", [P, P], F16, kind="ExternalInput")
    yt_out = nc.dram_tensor("yt_out", [GW, S], F32, kind="ExternalOutput")

    with tile.TileContext(nc) as tc:
        with ExitStack() as ctx:
            const = ctx.enter_context(tc.tile_pool(name="const", bufs=1))
            dram = ctx.enter_context(tc.tile_pool(name="dram", bufs=1, space="DRAM"))

            eps_t = const.tile([P, 1], F32, name="eps_t")
            nc.vector.memset(eps_t[:], EPS)
            neg1_t = const.tile([P, 1], F32, name="neg1_t")
            nc.vector.memset(neg1_t[:], -1.0)
            ones_h = const.tile([P, P], F16, name="ones_h")
            nc.vector.memset(ones_h[:], 1.0)
            ones_b = const.tile([P, P], mybir.dt.bfloat16, name="ones_b")
            nc.vector.memset(ones_b[:], 1.0)

            # plane 0: cos duplicated on both partition halves; plane 1:
            # +sin on rows 0..63, -sin on rows 64..127 (rope sign folded)
            cs_sb = const.tile([P, 2, S], F16, name="cs_sb")
            nc.sync.dma_start(out=cs_sb[:], in_=cs_in[:, :, :])
            mask_sb = const.tile([P, 4, 512], F16, name="mask_sb")
            nc.sync.dma_start(out=mask_sb[:], in_=mask_in.ap().rearrange("t p f -> p t f"))
            psw_sb = const.tile([P, P], F16, name="psw_sb")
            nc.scalar.dma_start(out=psw_sb[:], in_=psw_in[:, :])

            # attention chunks (q0, width)
            chunks = [(0, 512), (512, 512), (1024, 512), (1536, 512)]
            yt_ics = [
                dram.tile([GW, w], F16, name=f"yt_ic{i}")
                for i, (q0, w) in enumerate(chunks)
            ]
            ag_ics = [
                dram.tile([4 * GW, w], F16, name=f"ag_ic{i}")
                for i, (q0, w) in enumerate(chunks)
            ]

            # persistent SBUF
            wpool = ctx.enter_context(tc.tile_pool(name="wpool", bufs=1))
            wq_sb = wpool.tile([P, CT, GW], F16, name="wq_sb")
            wk_sb = wpool.tile([P, CT, GW], F16, name="wk_sb")
            wv_sb = wpool.tile([P, CT, GW], F16, name="wv_sb")
            wo_sb = wpool.tile([P, CT, GW], F16, name="wo_sb")
            qkv = ctx.enter_context(tc.tile_pool(name="qkv", bufs=1))
            qt_sb = qkv.tile([P, NHL, S], F16, name="qt_sb")
            kt_sb = qkv.tile([P, NHL, S], F16, name="kt_sb")
            v_sb = qkv.tile([P, CT, GW], F16, name="v_sb")

            # streaming pools (x in 256-token half-chunks)
            xtp = ctx.enter_context(tc.tile_pool(name="xtp", bufs=2))
            rawp = ctx.enter_context(tc.tile_pool(name="rawp", bufs=2))
            sqp = ctx.enter_context(tc.tile_pool(name="sqp", bufs=2))
            nrmp = ctx.enter_context(tc.tile_pool(name="nrmp", bufs=1))
            mp = ctx.enter_context(tc.tile_pool(name="mp", bufs=1))
            etp = ctx.enter_context(tc.tile_pool(name="etp", bufs=3))
            csp = ctx.enter_context(tc.tile_pool(name="csp", bufs=2))
            denp = ctx.enter_context(tc.tile_pool(name="denp", bufs=1))
            ytsp = ctx.enter_context(tc.tile_pool(name="ytsp", bufs=2))
            agp = ctx.enter_context(tc.tile_pool(name="agp", bufs=2))
            ysp = ctx.enter_context(tc.tile_pool(name="ysp", bufs=2))

            # PSUM: 2+2+2+2 = 8 banks
            proj_ps = ctx.enter_context(tc.tile_pool(name="proj_ps", bufs=2, space="PSUM"))
            s_ps = ctx.enter_context(tc.tile_pool(name="s_ps", bufs=2, space="PSUM"))
            yt_ps = ctx.enter_context(tc.tile_pool(name="yt_ps", bufs=2, space="PSUM"))
            bc_ps = ctx.enter_context(tc.tile_pool(name="bc_ps", bufs=2, space="PSUM"))

            # weight loads on scalar (x chunk 0 streams on sync in parallel)
            nc.scalar.dma_start(
                out=wq_sb[:], in_=wq_in.ap().rearrange("(a p) f -> p a f", p=P))
            nc.scalar.dma_start(
                out=wk_sb[:], in_=wk_in.ap().rearrange("(a p) f -> p a f", p=P))
            nc.scalar.dma_start(
                out=wv_sb[:], in_=wv_in.ap().rearrange("(a p) f -> p a f", p=P))
            nc.scalar.dma_start(
                out=wo_sb[:], in_=wo_in.ap().rearrange("(a p) f -> p a f", p=P))

            # tiny warm-up AllGather: pays the first-collective setup cost
            # and absorbs cross-core start skew off the critical path
            wu_d = dram.tile([1, 64], F16, name="wu_d")
            wu_o = dram.tile([4, 64], F16, name="wu_o")
            wu_sb = const.tile([1, 64], F16, name="wu_sb")
            nc.vector.memset(wu_sb[:], 0.0)
            nc.sync.dma_start(out=wu_d[:, :], in_=wu_sb[:])
            nc.gpsimd.collective_compute(
                "AllGather",
                mybir.AluOpType.bypass,
                replica_groups=[[0, 1, 2, 3], [4, 5, 6, 7]],
                ins=[wu_d[:].opt()],
                outs=[wu_o[:].opt()],
            )

            def emit_oproj(icc):
                q0, w = chunks[icc]
                ag_a = agp.tile([P, 8, 512], F16, name=f"ag_a{icc}", tag="ag")
                ag_b = agp.tile([P, 8, 512], F16, name=f"ag_b{icc}", tag="ag")
                for half, agt in ((0, ag_a), (1, ag_b)):
                    for m4 in range(2):
                        mt = half * 8 + m4 * 4
                        nc.sync.dma_start(
                            out=agt[:, m4 * 4:m4 * 4 + 4, 0:w],
                            in_=ag_ics[icc][mt * P:(mt + 4) * P, :]
                                .rearrange("(a p) f -> p a f", p=P),
                        )
                for oc in range(4):
                    yp = proj_ps.tile([P, w], F32, name=f"yp{icc}_{oc}", tag="proj")
                    for mt in range(CT):
                        agt = ag_a if mt < 8 else ag_b
                        nc.tensor.matmul(
                            yp[:],
                            wo_sb[:, mt, oc * P:(oc + 1) * P],
                            agt[:, mt % 8, 0:w],
                            start=(mt == 0), stop=(mt == CT - 1),
                        )
                    y_sb = ysp.tile([P, w], F32, name=f"ysb{icc}_{oc}", tag="ysb")
                    nc.scalar.copy(y_sb[:], yp[:])
                    nc.scalar.dma_start(
                        out=yt_out[oc * P:(oc + 1) * P, q0:q0 + w],
                        in_=y_sb[:],
                    )

            def emit_att(ci):
                q0, w = chunks[ci]
                njb = (q0 + w) // P
                jb0 = q0 // P
                for h in range(NHL):
                    ytp = yt_ps.tile([P, w], F32, name=f"yt{ci}_{h}", tag="yt")
                    csum = csp.tile(
                        [P, w], mybir.dt.bfloat16, name=f"cs{ci}_{h}", tag="cs")
                    ets = [None] * njb
                    for jb in range(njb):
                        sp = s_ps.tile([P, w], F32, name=f"s{ci}_{h}_{jb}", tag="s")
                        nc.tensor.matmul(
                            sp[:],
                            kt_sb[:, h, jb * P:(jb + 1) * P],
                            qt_sb[:, h, q0:q0 + w],
                            start=True, stop=True,
                        )
                        et = etp.tile([P, w], F16, name=f"et{ci}_{h}_{jb}", tag="et")
                        nc.scalar.activation(
                            et[:], sp[:],
                            mybir.ActivationFunctionType.Exp,
                            bias=neg1_t[:], scale=SCALE,
                        )
                        t = jb - jb0
                        if t >= 0:
                            nc.vector.tensor_mul(et[:], et[:], mask_sb[:, t, 0:w])
                        if jb == 0:
                            nc.vector.tensor_copy(csum[:], et[:])
                        else:
                            nc.vector.tensor_add(csum[:], csum[:], et[:])
                        ets[jb] = et
                        # AV lags the score by one tile so PE never waits
                        if jb >= 1:
                            nc.tensor.matmul(
                                ytp[:],
                                v_sb[:, jb - 1, h * HD:(h + 1) * HD],
                                ets[jb - 1][:],
                                start=(jb - 1 == 0), stop=False,
                            )
                    nc.tensor.matmul(
                        ytp[:],
                        v_sb[:, njb - 1, h * HD:(h + 1) * HD],
                        ets[njb - 1][:],
                        start=(njb == 1), stop=True,
                    )
                    den = bc_ps.tile([P, w], F32, name=f"den{ci}_{h}", tag="bc")
                    nc.tensor.matmul(
                        den[:], ones_b[:], csum[:], start=True, stop=True)
                    # rden = exp(-ln(den)) on the scalar engine (same ACT
                    # table set as the softmax exp)
                    lnd = denp.tile([P, w], F32, name=f"lnd{ci}_{h}", tag="lnd", bufs=1)
                    nc.scalar.activation(
                        lnd[:], den[:], mybir.ActivationFunctionType.Ln)
                    rden = denp.tile([P, w], F32, name=f"rdn{ci}_{h}", tag="rden")
                    nc.scalar.activation(
                        rden[:], lnd[:], mybir.ActivationFunctionType.Exp,
                        scale=-1.0)
                    yt_sb = ytsp.tile([P, w], F16, name=f"yts{ci}_{h}", tag="yts")
                    nc.vector.tensor_mul(yt_sb[:], ytp[:], rden[:])
                    nc.scalar.dma_start(
                        out=yt_ics[ci][h * P:(h + 1) * P, :], in_=yt_sb[:])

                nc.gpsimd.collective_compute(
                    "AllGather",
                    mybir.AluOpType.bypass,
                    replica_groups=[[0, 1, 2, 3], [4, 5, 6, 7]],
                    ins=[yt_ics[ci][:].opt()],
                    outs=[ag_ics[ci][:].opt()],
                )

            for tch in range(NTCH):
                tc0 = tch * 512
                xt_ch = xtp.tile([P, CT, 512], F16, name=f"xt{tch}", tag="xt")
                for c4 in range(4):
                    ct = c4 * 4
                    nc.sync.dma_start(
                        out=xt_ch[:, ct:ct + 4, :],
                        in_=xt_in[ct * P:(ct + 4) * P, tc0:tc0 + 512]
                            .rearrange("(a p) f -> p a f", p=P),
                    )

                # ---- Q then K: transposed projection + rms-norm + rope ----
                for wsb, dst, tag in ((wq_sb, qt_sb, "q"), (wk_sb, kt_sb, "k")):
                    raw4 = rawp.tile(
                        [P, NHL, 512], F16, name=f"{tag}raw{tch}", tag="raw")
                    nrm4 = nrmp.tile(
                        [P, NHL, 512], F16, name=f"{tag}nrm{tch}", tag="nrm")
                    sqs = []
                    for h in range(NHL):
                        ps = proj_ps.tile(
                            [P, 512], F32, name=f"{tag}ps{tch}_{h}", tag="proj")
                        for ct in range(CT):
                            nc.tensor.matmul(
                                ps[:],
                                wsb[:, ct, h * P:(h + 1) * P],
                                xt_ch[:, ct, :],
                                start=(ct == 0), stop=(ct == CT - 1),
                            )
                        nc.vector.tensor_copy(raw4[:, h, :], ps[:])
                        sq = sqp.tile(
                            [P, 512], F16, name=f"{tag}sq{tch}_{h}", tag="sq")
                        nc.vector.tensor_mul(sq[:], raw4[:, h, :], raw4[:, h, :])
                        sqs.append(sq)
                    for h in range(NHL):
                        ssum = bc_ps.tile(
                            [P, 512], F32, name=f"{tag}ss{tch}_{h}", tag="bc")
                        nc.tensor.matmul(
                            ssum[:], ones_h[:], sqs[h][:], start=True, stop=True)
                        # rstd = exp(-0.5*ln(ms+eps)) — Ln and Exp share
                        # one ACT table set, so no table switches
                        lnt = sqp.tile(
                            [P, 512], F16, name=f"{tag}ln{tch}_{h}", tag="lnt")
                        nc.scalar.activation(
                            lnt[:], ssum[:],
                            mybir.ActivationFunctionType.Ln,
                            bias=eps_t[:], scale=1.0 / HD,
                        )
                        nc.scalar.activation(
                            nrm4[:, h, :], lnt[:],
                            mybir.ActivationFunctionType.Exp,
                            scale=-0.5,
                        )
                    # rope: m1 = raw*cos_dup; m2 = swap(raw)*sin_signed
                    # (swap = partition rotation by 64 via PE permutation
                    # matmul); dst = (m1 + m2) * rstd
                    cosB = cs_sb[:, 0:1, tc0:tc0 + 512].broadcast_to((P, 1, 512))
                    sinB = cs_sb[:, 1:2, tc0:tc0 + 512].broadcast_to((P, 1, 512))
                    for g in range(NHL):
                        hs = slice(g, g + 1)
                        qs_ps = s_ps.tile(
                            [P, 512], F32, name=f"{tag}qsw{tch}_{g}", tag="s")
                        nc.tensor.matmul(
                            qs_ps[:], psw_sb[:],
                            raw4[:, g, :], start=True, stop=True)
                        qsv = qs_ps[:].rearrange("p (h f) -> p h f", h=1)
                        m1 = mp.tile(
                            [P, 1, 512], F16, name=f"{tag}m1{tch}_{g}", tag="m1")
                        m2 = mp.tile(
                            [P, 1, 512], F16, name=f"{tag}m2{tch}_{g}", tag="m2")
                        nc.vector.tensor_mul(m1[:], raw4[:, hs, :], cosB)
                        nc.vector.tensor_mul(m2[:], qsv, sinB)
                        nc.vector.tensor_add(m1[:], m1[:], m2[:])
                        nc.vector.tensor_mul(
                            dst[:, hs, tc0:tc0 + 512], m1[:], nrm4[:, hs, :])

                # ---- V: row-layout projection ----
                for ib in range(4):
                    jb = tch * 4 + ib
                    ps = proj_ps.tile([P, GW], F32, name=f"vps{jb}", tag="proj")
                    for ct in range(CT):
                        nc.tensor.matmul(
                            ps[:],
                            xt_ch[:, ct, ib * P:(ib + 1) * P],
                            wv_sb[:, ct, :],
                            start=(ct == 0), stop=(ct == CT - 1),
                        )
                    nc.vector.tensor_copy(v_sb[:, jb, :], ps[:])

                emit_att(tch)
                if tch == 2:
                    emit_oproj(0)

            emit_oproj(1)
            emit_oproj(2)
            emit_oproj(3)

    nc.compile()
    _program_cache["nc"] = nc
    return nc


def _rope_tables():
    inv_freq = 1.0 / (10000.0 ** (np.arange(0, HD, 2, dtype=np.float32) / HD))
    pos = np.arange(S, dtype=np.float32)
    freqs = np.outer(pos, inv_freq).astype(np.float32)  # [S, 64]
    cosT = np.cos(freqs).T  # [64, S]
    sinT = np.sin(freqs).T
    cs = np.empty((P, 2, S), dtype=np.float16)
    cs[0:64, 0] = cosT
    cs[64:128, 0] = cosT
    cs[0:64, 1] = sinT
    cs[64:128, 1] = -sinT
    return cs


def _mask_tiles():
    m = np.zeros((4, P, 512), dtype=np.float16)
    jj = np.arange(P)[:, None]
    ii = np.arange(512)[None, :]
    for t in range(4):
        m[t] = np.where(t * P + jj > ii, 0.0, 1.0)
    return m


def make_in_maps(x, Wq, Wk, Wv, Wo):
    x = np.asarray(x, dtype=np.float32)
    cs = _rope_tables()
    maskt = _mask_tiles()
    wqT = np.ascontiguousarray(np.asarray(Wq, dtype=np.float32).T.astype(np.float16))
    wkT = np.ascontiguousarray(np.asarray(Wk, dtype=np.float32).T.astype(np.float16))
    wvT = np.ascontiguousarray(np.asarray(Wv, dtype=np.float32).T.astype(np.float16))
    woT = np.ascontiguousarray(np.asarray(Wo, dtype=np.float32).T.astype(np.float16))
    xts = [np.ascontiguousarray(x[b].T.astype(np.float16)) for b in range(2)]
    psw = np.zeros((P, P), dtype=np.float16)
    kk = np.arange(P)
    psw[(kk + 64) % P, kk] = 1.0
    in_maps = []
    for c in range(NCORES):
        b, g = c // 4, c % 4
        sl = slice(g * GW, (g + 1) * GW)
        in_maps.append({
            "xt": xts[b],
            "wq": np.ascontiguousarray(wqT[:, sl]),
            "wk": np.ascontiguousarray(wkT[:, sl]),
            "wv": np.ascontiguousarray(wvT[:, sl]),
            "wo": np.ascontiguousarray(woT[:, sl]),
            "cs": cs,
            "maskt": maskt,
            "psw": psw,
        })
    return in_maps


def assemble_output(results):
    y = np.empty((2, S, D), dtype=np.float32)
    for c in range(NCORES):
        b, g = c // 4, c % 4
        y[b][:, g * GW:(g + 1) * GW] = results[c]["yt_out"].T
    return y


def kernel(x, Wq, Wk, Wv, Wo):
    nc = build_program()
    in_maps = make_in_maps(x, Wq, Wk, Wv, Wo)
    res = run_bass_kernel_spmd(nc, in_maps, core_ids=list(range(NCORES)))
    return assemble_output(res.results)
